# revision 1
# baseline (speedup 1.0000x reference)
"""Trainium2 Bass kernel for nn_DAWN_41549513621652.

Strategy (8 NeuronCores, single chip, no cross-core collectives):
  The model's heavy compute is dense matmul (attention, Wo, memory WV,
  lm_head). The glue (layernorm, the 512-step SSM scan, routing softmax,
  and the DMA-bound neuron-pool contractions nw@{comp,EQ,EK,EV}) is tiny
  FLOP-wise and runs on host between device launches; host also performs
  the cross-core reductions (summing Wo partials) so the device programs
  need no collectives.

  5 device launches per call:
    A (x2): circuit module, head-sharded — core c owns heads {2c, 2c+1}
            for both batch elements; outputs per-core Wo partials.
    C (x2): memory module, token-sharded — core c owns 128 tokens
            (b=c//4, s in [128*(c%4), ...)); exact top-16 via DVE
            max8/match_replace threshold, dense masked-softmax, PE WV.
    D (x1): lm_head, vocab-sharded — core c owns a 4096-wide slice of the
            zero-padded 32768 vocab.

  Everything is fp32: the memory module's top-16 selection has score gaps
  down to 7e-9, so any lower precision upstream flips selections vs the
  reference.
"""

import numpy as np

import concourse.bass as bass
import concourse.bacc as bacc
import concourse.mybir as mybir
import concourse.tile as tile
from concourse.bass_utils import run_bass_kernel_spmd
from concourse.masks import make_identity

F32 = mybir.dt.float32
F32R = mybir.dt.float32r


def _mmr(nc, out, lhsT, rhs, **kw):
    """float32r matmul (operand tiles are already float32r-typed)."""
    nc.tensor.matmul(out, lhsT, rhs, **kw)

# model dims (hardcoded per problem spec)
L, D, H, R, NC, NK, KK, SD, V, B, S = 2, 1024, 16, 128, 64, 1024, 16, 64, 32000, 2, 512
DH = D // H          # 64
T = B * S            # 1024
N_CORES = 8
VP = 32768           # padded vocab
VSL = VP // N_CORES  # 4096 per-core vocab slice
DT = D // 128        # 8 d-tiles
NEG = -1e30


# ---------------------------------------------------------------- device programs


def _build_A(n_iter: int = 1):
    """Circuit module. Per-core inputs:
      xnT  [D, T]    d-major normalized activations (cols = b*S+s)
      sc   [B, D, R] dynamic compress basis (host: nw@comp)
      eqs/eks/evs [B, R, 128]  expansion slices for this core's 2 heads
      woT  [128, D]  o_w.T rows for this core's d_in slice
      tri  [128, 128] upper-tri (incl diag) causal mask for scoresT layout
    Output:
      part [B, D, S] Wo partial, d-major
    """
    nc = bacc.Bacc("TRN2", target_bir_lowering=False, debug=False,
                   num_devices=N_CORES)
    xnT_d = nc.dram_tensor("xnT", [D, T], F32R, kind="ExternalInput")
    sc_d = nc.dram_tensor("sc", [B, D, R], F32R, kind="ExternalInput")
    eqs_d = nc.dram_tensor("eqs", [B, R, 128], F32R, kind="ExternalInput")
    eks_d = nc.dram_tensor("eks", [B, R, 128], F32R, kind="ExternalInput")
    evs_d = nc.dram_tensor("evs", [B, R, 128], F32R, kind="ExternalInput")
    woT_d = nc.dram_tensor("woT", [128, D], F32R, kind="ExternalInput")
    tri_d = nc.dram_tensor("tri", [128, 128], F32R, kind="ExternalInput")
    part_d = nc.dram_tensor("part", [B, D, S], F32, kind="ExternalOutput")

    with tile.TileContext(nc) as tc:
        with (
            tc.tile_pool(name="big", bufs=1) as big,
            tc.tile_pool(name="work", bufs=3) as work,
            tc.tile_pool(name="small", bufs=2) as small,
            tc.tile_pool(name="ps", bufs=2, space="PSUM") as ps,
            tc.tile_pool(name="ps1", bufs=2, space="PSUM") as ps1,
            tc.tile_pool(name="out", bufs=3) as outp,
        ):
            def body(_it):
                xn = big.tile([128, DT, T], F32R, tag="xn")
                nc.sync.dma_start(xn[:], xnT_d.ap().rearrange("(dt p) t -> p dt t", p=128))
                sc = big.tile([128, B, DT, R], F32R, tag="sc")
                nc.scalar.dma_start(sc[:], sc_d.ap().rearrange("b (dt p) r -> p b dt r", p=128))
                eq = big.tile([128, B, 128], F32R, tag="eq")
                ek = big.tile([128, B, 128], F32R, tag="ek")
                ev = big.tile([128, B, 128], F32R, tag="ev")
                nc.scalar.dma_start(eq[:], eqs_d.ap().rearrange("b r e -> r b e"))
                nc.scalar.dma_start(ek[:], eks_d.ap().rearrange("b r e -> r b e"))
                nc.scalar.dma_start(ev[:], evs_d.ap().rearrange("b r e -> r b e"))
                wo = big.tile([128, D], F32R, tag="wo")
                nc.scalar.dma_start(wo[:], woT_d.ap())
                tri = big.tile([128, 128], F32R, tag="tri")
                nc.scalar.dma_start(tri[:], tri_d.ap())
                ones = big.tile([128, 1], F32R, tag="ones")
                nc.vector.memset(ones[:].bitcast(F32), 1.0)
                ones_row = big.tile([1, 64], F32, tag="ones_row")
                nc.vector.memset(ones_row[:], 1.0)

                # hT[b] [R=128, S] = sc[b].T @ xnT[b]
                h = big.tile([128, B, S], F32R, tag="h")
                for b in range(B):
                    hp = ps.tile([128, S], F32, tag="mm")
                    for dt in range(DT):
                        _mmr(nc, hp[:], sc[:, b, dt, :], xn[:, dt, b * S:(b + 1) * S],
                                         start=(dt == 0), stop=(dt == DT - 1))
                    nc.vector.tensor_copy(h[:, b, :], hp[:])

                # QT/KT [128(dh2), B, S]; V token-major [128(tok), B, 4, 128(dh2)]
                qt = big.tile([128, B, S], F32R, tag="qt")
                kt_ = big.tile([128, B, S], F32R, tag="kt")
                vt = big.tile([128, B, 4, 128], F32R, tag="vt")
                for b in range(B):
                    qp = ps.tile([128, S], F32, tag="mm")
                    _mmr(nc, qp[:], eq[:, b, :], h[:, b, :])
                    nc.vector.tensor_copy(qt[:, b, :], qp[:])
                    kp = ps.tile([128, S], F32, tag="mm")
                    _mmr(nc, kp[:], ek[:, b, :], h[:, b, :])
                    nc.vector.tensor_copy(kt_[:, b, :], kp[:])
                    for st in range(4):
                        vp = ps1.tile([128, 512], F32, tag="aux1", name="vp")[:, :128]
                        _mmr(nc, vp[:], h[:, b, st * 128:(st + 1) * 128], ev[:, b, :])
                        nc.vector.tensor_copy(vt[:, b, st, :], vp[:])

                # attention per (b, head-in-core)
                att = big.tile([128, B, S], F32R, tag="att")  # [d_in(2*64), b, q]
                for b in range(B):
                    for hh in range(2):
                        p0 = 64 * hh
                        et = work.tile([128, 4, S], F32R, tag="et")
                        for kt in range(4):
                            q0 = 128 * kt
                            sp = ps.tile([128, S], F32, tag="sp")
                            _mmr(nc, 
                                sp[:, q0:S],
                                kt_[p0:p0 + 64, b, kt * 128:(kt + 1) * 128],
                                qt[p0:p0 + 64, b, q0:S])
                            # e = exp(s / sqrt(DH)), causal-masked on the diagonal block
                            nc.scalar.activation(et[:, kt, q0:S], sp[:, q0:S],
                                                 mybir.ActivationFunctionType.Exp,
                                                 scale=float(1.0 / np.sqrt(DH)))
                            nc.vector.tensor_mul(et[:, kt, q0:q0 + 128],
                                                 et[:, kt, q0:q0 + 128], tri[:])
                        # Z[q] = sum_k e[k,q] via ones-matmul; then 1/Z
                        zp = ps1.tile([128, S], F32, tag="aux1", name="zp")[:1, :]
                        for kt in range(4):
                            _mmr(nc, zp[:, 128 * kt:S], ones[:],
                                             et[:, kt, 128 * kt:S],
                                             start=(kt == 0), stop=(kt == 3))
                        zr = small.tile([1, S], F32, tag="zr")
                        nc.vector.reciprocal(zr[:], zp[:])
                        zbp = ps1.tile([128, S], F32, tag="aux1", name="zbp")[:64, :]
                        _mmr(nc, zbp[:], ones_row[:], zr[:])
                        zb = small.tile([64, S], F32R, tag="zb")
                        nc.vector.tensor_copy(zb[:], zbp[:])
                        # out_hT [64(dh), S] = sum_k V[k,dh].T-form @ e[k,q]
                        op_full = ps.tile([128, S], F32, tag="op", name="op")
                        op = op_full[:64, :]
                        for kt in range(4):
                            _mmr(nc, op[:, 128 * kt:S],
                                             vt[:, b, kt, p0:p0 + 64],
                                             et[:, kt, 128 * kt:S],
                                             start=(kt == 0), stop=(kt == 3))
                        nc.vector.tensor_mul(att[p0:p0 + 64, b, :], op[:], zb[:])

                # Wo partial: part[b].T [d_out, S] = woT.T @ att[b]
                for b in range(B):
                    for mt in range(DT):
                        wp = ps.tile([128, S], F32, tag="mm")
                        _mmr(nc, wp[:], wo[:, mt * 128:(mt + 1) * 128], att[:, b, :])
                        ot = outp.tile([128, S], F32, tag="ot")
                        nc.vector.tensor_copy(ot[:], wp[:])
                        eng = nc.scalar if mt % 2 else nc.sync
                        eng.dma_start(
                            part_d.ap()[b, mt * 128:(mt + 1) * 128, :], ot[:])

            if n_iter == 1:
                body(0)
            else:
                with tc.For_i(0, n_iter, 1) as it:
                    body(it)
    nc.compile()
    return nc


def _build_C(n_iter: int = 1):
    """Memory module, token-sharded (128 tokens per core). Inputs:
      xnTs [D, 128]  d-major xn columns for this core's tokens
      scb  [D, R]    compress basis for this core's batch element
      kKT  [R, NK]   knowledge_K.T
      kV   [NK, D]
    Output: mo [128, D] memory output rows for this core's tokens."""
    nc = bacc.Bacc("TRN2", target_bir_lowering=False, debug=False,
                   num_devices=N_CORES)
    xn_d = nc.dram_tensor("xnTs", [D, 128], F32, kind="ExternalInput")
    sc_d = nc.dram_tensor("scb", [D, R], F32, kind="ExternalInput")
    kk_d = nc.dram_tensor("kKT", [R, NK], F32, kind="ExternalInput")
    kv_d = nc.dram_tensor("kV", [NK, D], F32R, kind="ExternalInput")
    mo_d = nc.dram_tensor("mo", [128, D], F32, kind="ExternalOutput")
    NT = NK // 128  # 8
    inv_sqrt_r = float(1.0 / np.sqrt(R))

    with tile.TileContext(nc) as tc:
        with (
            tc.tile_pool(name="big", bufs=1) as big,
            tc.tile_pool(name="work", bufs=2) as work,
            tc.tile_pool(name="ps", bufs=2, space="PSUM") as ps,
            tc.tile_pool(name="ps1", bufs=2, space="PSUM") as ps1,
        ):
            def body(_it):
                xn = big.tile([128, DT, 128], F32, tag="xn")
                nc.sync.dma_start(xn[:], xn_d.ap().rearrange("(dt p) t -> p dt t", p=128))
                sc = big.tile([128, DT, R], F32, tag="sc")
                nc.sync.dma_start(sc[:], sc_d.ap().rearrange("(dt p) r -> p dt r", p=128))
                kk = big.tile([128, NK], F32, tag="kk")
                nc.sync.dma_start(kk[:], kk_d.ap())
                kv = big.tile([128, NT, D], F32R, tag="kv")
                nc.sync.dma_start(kv[:], kv_d.ap().rearrange("(nt p) d -> p nt d", p=128))

                # QT [R, tok]
                qp_full = ps.tile([128, 512], F32, tag="mm", name="qp")
                qp = qp_full[:, :128]
                for dt in range(DT):
                    nc.tensor.matmul(qp[:], sc[:, dt, :], xn[:, dt, :],
                                     start=(dt == 0), stop=(dt == DT - 1))
                q = work.tile([128, 128], F32, tag="q")
                nc.vector.tensor_copy(q[:], qp[:])

                # scores token-major [tok, NK] (scaled)
                s = work.tile([128, NK], F32, tag="s")
                for c2 in range(2):
                    sp = ps.tile([128, 512], F32, tag="mm")
                    nc.tensor.matmul(sp[:], q[:], kk[:, c2 * 512:(c2 + 1) * 512])
                    nc.vector.tensor_scalar_mul(s[:, c2 * 512:(c2 + 1) * 512], sp[:],
                                                inv_sqrt_r)
                # scoresT [nk, tok] (scaled)
                st = work.tile([128, NT, 128], F32, tag="st")
                for nt in range(NT):
                    tp = ps1.tile([128, 128], F32, tag="aux")
                    nc.tensor.matmul(tp[:], kk[:, nt * 128:(nt + 1) * 128], q[:])
                    nc.vector.tensor_scalar_mul(st[:, nt, :], tp[:], inv_sqrt_r)

                # top-16 threshold per token row: tau strictly between 16th/17th
                m8a = work.tile([128, 8], F32, tag="m8a")
                m8b = work.tile([128, 8], F32, tag="m8b")
                m8c = work.tile([128, 8], F32, tag="m8c")
                s2 = work.tile([128, NK], F32, tag="s2")
                s3 = work.tile([128, NK], F32, tag="s3")
                nc.vector.max(m8a[:], s[:])
                nc.vector.match_replace(s2[:], m8a[:], s[:], NEG)
                nc.vector.max(m8b[:], s2[:])
                nc.vector.match_replace(s3[:], m8b[:], s2[:], NEG)
                nc.vector.max(m8c[:], s3[:])
                tau = work.tile([128, 1], F32, tag="tau")
                nc.vector.tensor_add(tau[:], m8b[:, 7:8], m8c[:, 0:1])
                nc.vector.tensor_scalar_mul(tau[:], tau[:], 0.5)
                negm = work.tile([128, 1], F32, tag="negm")
                nc.vector.tensor_scalar_mul(negm[:], m8a[:, 0:1], -1.0)

                # Z per token from token-major layout (per-partition stats)
                etok = work.tile([128, NK], F32, tag="etok")
                nc.scalar.activation(etok[:], s[:], mybir.ActivationFunctionType.Exp,
                                     bias=negm[:])
                msk = work.tile([128, NK], F32, tag="msk")
                nc.vector.tensor_scalar(msk[:], s[:], tau[:], scalar2=None,
                                        op0=mybir.AluOpType.is_gt)
                nc.vector.tensor_mul(etok[:], etok[:], msk[:])
                z = work.tile([128, 1], F32, tag="z")
                nc.vector.reduce_sum(z[:], etok[:], axis=mybir.AxisListType.X)
                zr = work.tile([128, 1], F32, tag="zr")
                nc.vector.reciprocal(zr[:], z[:])

                # stats broadcast to T-layout (tok along free dim):
                # transpose [128,1] -> [1,128], then ones-column outer product
                idn = big.tile([128, 128], F32, tag="idn")
                make_identity(nc, idn[:])
                ones_row = work.tile([1, 128], F32, tag="ones_row")
                nc.vector.memset(ones_row[:], 1.0)

                def bcast_T(col, nm):
                    rp_f = ps1.tile([128, 128], F32, tag="aux", name=f"{nm}_rp")
                    rp = rp_f[:1, :]
                    nc.tensor.transpose(rp[:], col[:], idn[:])
                    row = work.tile([1, 128], F32, tag=f"{nm}_row", name=f"{nm}_row")
                    nc.vector.tensor_copy(row[:], rp[:])
                    bp = ps1.tile([128, 128], F32, tag="aux", name=f"{nm}_bp")
                    nc.tensor.matmul(bp[:], ones_row[:], row[:])
                    bc = work.tile([128, 128], F32, tag=f"{nm}_bc", name=f"{nm}_bc")
                    nc.vector.tensor_copy(bc[:], bp[:])
                    return bc

                negmT = bcast_T(negm, "negm")
                tauT = bcast_T(tau, "tau")

                # masked exp in T-layout, then WV matmul
                et = work.tile([128, NT, 128], F32R, tag="et")
                for nt in range(NT):
                    nc.vector.tensor_add(et[:, nt, :], st[:, nt, :], negmT[:])
                    nc.scalar.activation(et[:, nt, :], et[:, nt, :],
                                         mybir.ActivationFunctionType.Exp)
                    mk = work.tile([128, 128], F32, tag="mk")
                    nc.vector.tensor_tensor(mk[:], st[:, nt, :], tauT[:],
                                            op=mybir.AluOpType.is_gt)
                    nc.vector.tensor_mul(et[:, nt, :], et[:, nt, :], mk[:])

                out = work.tile([128, D], F32, tag="out")
                for c2 in range(2):
                    op = ps.tile([128, 512], F32, tag="mm")
                    for nt in range(NT):
                        _mmr(nc, op[:], et[:, nt, :],
                             kv[:, nt, c2 * 512:(c2 + 1) * 512],
                             start=(nt == 0), stop=(nt == NT - 1))
                    nc.vector.tensor_scalar_mul(out[:, c2 * 512:(c2 + 1) * 512],
                                                op[:], zr[:])
                nc.scalar.dma_start(mo_d.ap(), out[:])

            if n_iter == 1:
                body(0)
            else:
                with tc.For_i(0, n_iter, 1) as it:
                    body(it)
    nc.compile()
    return nc


def _build_D(n_iter: int = 1):
    """lm_head, vocab-sharded. Inputs: xfT [D, T]; hwT [D, VSL].
    Output: lo [T, VSL]."""
    nc = bacc.Bacc("TRN2", target_bir_lowering=False, debug=False,
                   num_devices=N_CORES)
    xf_d = nc.dram_tensor("xfT", [D, T], F32R, kind="ExternalInput")
    hw_d = nc.dram_tensor("hwT", [D, VSL], F32R, kind="ExternalInput")
    lo_d = nc.dram_tensor("lo", [T, VSL], F32, kind="ExternalOutput")
    NVC = VSL // 512  # 8 chunks

    with tile.TileContext(nc) as tc:
        with (
            tc.tile_pool(name="big", bufs=1) as big,
            tc.tile_pool(name="wpool", bufs=4) as wpool,
            tc.tile_pool(name="opool", bufs=6) as opool,
            tc.tile_pool(name="ps", bufs=8, space="PSUM") as ps,
        ):
            def body(_it):
                xf = big.tile([128, DT, T], F32R, tag="xf")
                nc.sync.dma_start(xf[:], xf_d.ap().rearrange("(dt p) t -> p dt t", p=128))
                for vc in range(NVC):
                    hw = wpool.tile([128, DT, 512], F32R, tag="hw")
                    nc.sync.dma_start(
                        hw[:], hw_d.ap()[:, vc * 512:(vc + 1) * 512]
                        .rearrange("(dt p) v -> p dt v", p=128))
                    for tt in range(DT):
                        pp = ps.tile([128, 512], F32, tag="pp")
                        for dt in range(DT):
                            _mmr(nc, pp[:], xf[:, dt, tt * 128:(tt + 1) * 128],
                                             hw[:, dt, :],
                                             start=(dt == 0), stop=(dt == DT - 1))
                        ot = opool.tile([128, 512], F32, tag="ot")
                        nc.vector.tensor_copy(ot[:], pp[:])
                        eng = nc.scalar if tt % 2 else nc.sync
                        eng.dma_start(
                            lo_d.ap()[tt * 128:(tt + 1) * 128,
                                      vc * 512:(vc + 1) * 512], ot[:])

            if n_iter == 1:
                body(0)
            else:
                with tc.For_i(0, n_iter, 1) as it:
                    body(it)
    nc.compile()
    return nc


_PROGS = {}


def _prog(name, n_iter=1):
    key = (name, n_iter)
    if key not in _PROGS:
        _PROGS[key] = {"A": _build_A, "C": _build_C, "D": _build_D}[name](n_iter)
    return _PROGS[key]


# ---------------------------------------------------------------- host-side math


def _ln(x, w, b):
    m = x.mean(-1, keepdims=True, dtype=np.float32)
    v = ((x - m) ** 2).mean(-1, keepdims=True, dtype=np.float32)
    return ((x - m) / np.sqrt(v + np.float32(1e-5)) * w + b).astype(np.float32)


def _softmax(x, axis=-1):
    m = x.max(axis=axis, keepdims=True)
    e = np.exp(x - m)
    return e / e.sum(axis=axis, keepdims=True)


def _nw(xn, A, Bm, Wimp, Wr):
    """SSM scan + routing -> neuron weights [B, NC] (host, fp32)."""
    u = xn @ Bm                       # [B,S,SD]
    h = np.zeros((xn.shape[0], A.shape[0]), np.float32)
    for t in range(xn.shape[1]):
        h = h @ A + u[:, t]
    h_proj = h @ Wimp.T               # [B, D]
    imp = _softmax(np.einsum('bsd,bd->bs', xn, h_proj), axis=-1)
    pref = _softmax(xn @ Wr.T, axis=-1)
    nw = np.einsum('bs,bsn->bn', imp, pref)
    return (nw / (nw.sum(-1, keepdims=True) + np.float32(1e-8))).astype(np.float32)


def _pack_T(x):
    """[B,S,D] -> d-major [D, B*S] fp32 contiguous."""
    return np.ascontiguousarray(
        np.concatenate([x[b].T for b in range(B)], axis=1), dtype=np.float32)


_run_ncores = list(range(N_CORES))
_LAST_MAPS = {}


def _run(name, in_maps):
    _LAST_MAPS[name] = in_maps
    res = run_bass_kernel_spmd(_prog(name), in_maps, core_ids=_run_ncores)
    return res.results


def kernel(**inputs) -> np.ndarray:
    inp = {k: np.asarray(v) for k, v in inputs.items()}
    ids = inp['input_ids'].astype(np.int64)
    comp_f = inp['compress_neurons'].reshape(NC, -1).astype(np.float32)
    tri = np.triu(np.ones((128, 128), np.float32))
    kKT = np.ascontiguousarray(inp['knowledge_K'].T, dtype=np.float32)
    kV = np.ascontiguousarray(inp['knowledge_V'], dtype=np.float32)

    x = (inp['tok_emb'][ids] + inp['pos_emb'][None, :ids.shape[1]]).astype(np.float32)

    for l in range(L):
        # ---- circuit (device program A, head-sharded) ----
        xn = _ln(x, inp['ln1_w'][l], inp['ln1_b'][l])
        nw = _nw(xn, inp['a_A'][l], inp['a_B'][l], inp['a_imp'][l], inp['a_router'][l])
        sc = (nw @ comp_f).reshape(B, D, R)
        eq = (nw @ inp['eQ'][l].reshape(NC, -1).astype(np.float32)).reshape(B, R, D)
        ek = (nw @ inp['eK'][l].reshape(NC, -1).astype(np.float32)).reshape(B, R, D)
        ev = (nw @ inp['eV'][l].reshape(NC, -1).astype(np.float32)).reshape(B, R, D)
        woT = np.ascontiguousarray(inp['o_w'][l].T, dtype=np.float32)
        xnT = _pack_T(xn)
        in_maps = []
        for c in range(N_CORES):
            sl = slice(128 * c, 128 * (c + 1))
            in_maps.append({
                "xnT": xnT,
                "sc": np.ascontiguousarray(sc, dtype=np.float32),
                "eqs": np.ascontiguousarray(eq[:, :, sl]),
                "eks": np.ascontiguousarray(ek[:, :, sl]),
                "evs": np.ascontiguousarray(ev[:, :, sl]),
                "woT": np.ascontiguousarray(woT[sl, :]),
                "tri": tri,
            })
        res = _run("A", in_maps)
        circT = res[0]["part"]
        for c in range(1, N_CORES):
            circT = circT + res[c]["part"]
        x = x + circT.transpose(0, 2, 1)

        # ---- memory (device program C, token-sharded) ----
        xn = _ln(x, inp['ln2_w'][l], inp['ln2_b'][l])
        nw = _nw(xn, inp['m_A'][l], inp['m_B'][l], inp['m_imp'][l], inp['m_router'][l])
        sc = (nw @ comp_f).reshape(B, D, R)
        in_maps = []
        for c in range(N_CORES):
            bc, s0 = c // 4, 128 * (c % 4)
            in_maps.append({
                "xnTs": np.ascontiguousarray(xn[bc, s0:s0 + 128, :].T),
                "scb": np.ascontiguousarray(sc[bc]),
                "kKT": kKT,
                "kV": kV,
            })
        res = _run("C", in_maps)
        mo = np.empty((B, S, D), np.float32)
        for c in range(N_CORES):
            bc, s0 = c // 4, 128 * (c % 4)
            mo[bc, s0:s0 + 128] = res[c]["mo"]
        x = x + mo

    # ---- lm_head (device program D, vocab-sharded) ----
    xf = _ln(x, inp['lnf_w'], inp['lnf_b'])
    xfT = _pack_T(xf)
    hwT = np.zeros((D, VP), np.float32)
    hwT[:, :V] = inp['head_w'].astype(np.float32).T
    in_maps = [{"xfT": xfT,
                "hwT": np.ascontiguousarray(hwT[:, VSL * c:VSL * (c + 1)])}
               for c in range(N_CORES)]
    res = _run("D", in_maps)
    logits = np.concatenate([res[c]["lo"] for c in range(N_CORES)], axis=1)
    return logits[:, :V].reshape(B, S, V)



# revision 51
# speedup vs baseline: 1.6722x; 1.6722x over previous
"""Trainium2 Bass kernel for nn_DAWN_41549513621652.

Strategy (8 NeuronCores, single chip, no cross-core collectives):
  Dense matmul work (attention circuit, memory WV, lm_head) runs on device;
  scalar glue (layernorm, the 512-step SSM scan, routing softmax, neuron-pool
  contractions, partial-sum reductions) runs on host between launches.

  5 device launches per call:
    A  (x2): circuit, sharded (batch x 4-head-group) per core. fp16
             internals (validated: attention-internal quantization does not
             perturb the residual stream enough to flip the memory module's
             top-16 selection), fp32 att->Wo path and fp32 partial outputs.
    C1/C2  : memory module, token-sharded (128 tokens/core). The top-16
             selection path (Q, scores, max8 chain) is exact fp32 with the
             same matmul shapes/order as the reference-validated baseline.
             Selection uses RAW scores (scale-invariant order) with an
             is_ge-16th-value threshold. Layer 1's value path stays fp32
             (its output feeds layer 2's selection); layer 2's kV/exp run
             fp16 (only feeds the logits).
    D  (x1): lm_head, vocab-sharded 4000 cols/core, all fp16 (validated
             2.7e-4 rel err). Weights streamed and double-buffered behind
             the PE, which is the roofline engine (~107us of fp32-accum
             fp16 matmul per core).
"""

import numpy as np

import concourse.bass as bass
import concourse.bacc as bacc
import concourse.mybir as mybir
import concourse.tile as tile
from concourse.bass_utils import run_bass_kernel_spmd
from concourse.masks import make_identity

F32 = mybir.dt.float32
F32R = mybir.dt.float32r
F16 = mybir.dt.float16

# model dims (hardcoded per problem spec)
L, D, H, R, NC, NK, KK, SD, V, B, S = 2, 1024, 16, 128, 64, 1024, 16, 64, 32000, 2, 512
DH = D // H          # 64
T = B * S            # 1024
N_CORES = 8
VSL = V // N_CORES   # 4000 per-core vocab slice
NVC = 8              # vocab chunks per core
VC = VSL // NVC      # 500 cols per chunk
DT = D // 128        # 8 d-tiles
NT = NK // 128       # 8 knowledge tiles
NEG = -1e30
EXP = mybir.ActivationFunctionType.Exp
COPY = mybir.ActivationFunctionType.Copy


# ---------------------------------------------------------------- device programs


def _build_A():
    """Circuit attention; core owns ONE batch element and FOUR heads. The
    tiny low-rank h/Q/K/V expansions (~0.2 GFLOP) are host-side glue.
    Inputs (host-prearranged partition-major):
      qt/kt [128, 2, S] f16    Q.T / K.T for this core's 4 heads
      vt  [128, 4, 4, 65] f16  V token-major per (st, hh), ones col baked in
      wo  [128, 2, D] f32r     o_w.T rows for this core's 256-wide d_in slice
      tri [128, 128] f16       upper-tri (incl diag) causal mask
    Output:
      part [DT, 128, S] f16  Wo partial for this core's batch, d-major tiles
    """
    nc = bacc.Bacc("TRN2", target_bir_lowering=False, debug=False,
                   num_devices=N_CORES)
    qt_d = nc.dram_tensor("qt", [128, 2, S], F16, kind="ExternalInput")
    kt_d = nc.dram_tensor("kt", [128, 2, S], F16, kind="ExternalInput")
    vt_d = nc.dram_tensor("vt", [128, 4, 4, 65], F16, kind="ExternalInput")
    wo_d = nc.dram_tensor("wo", [128, 2, D], F32R, kind="ExternalInput")
    tri_d = nc.dram_tensor("tri", [128, 128], F16, kind="ExternalInput")
    part_d = nc.dram_tensor("part", [DT, 128, S], F16, kind="ExternalOutput")
    scale = float(1.0 / np.sqrt(DH))

    with tile.TileContext(nc) as tc:
        with (
            tc.tile_pool(name="big", bufs=1) as big,
            tc.tile_pool(name="etp", bufs=4) as etp,
            tc.tile_pool(name="ps", bufs=3, space="PSUM") as ps,
            tc.tile_pool(name="psb", bufs=2, space="PSUM") as psb,
            tc.tile_pool(name="psz", bufs=2, space="PSUM") as psz,
            tc.tile_pool(name="out", bufs=3) as outp,
        ):
            # inputs: host precomputes the h/Q/K/V low-rank expansions
            qt = big.tile([128, 2, S], F16, tag="qt")
            nc.sync.dma_start(qt[:], qt_d.ap())
            kt_ = big.tile([128, 2, S], F16, tag="kt")
            nc.sync.dma_start(kt_[:], kt_d.ap())
            vt = big.tile([128, 4, 4, 65], F16, tag="vt")
            nc.sync.dma_start(vt[:], vt_d.ap())
            tri = big.tile([128, 128], F16, tag="tri")
            nc.sync.dma_start(tri[:], tri_d.ap())
            wo = big.tile([128, 2, D], F32R, tag="wo")
            nc.sync.dma_start(wo[:], wo_d.ap())

            ones_col = big.tile([1, 64], F16, tag="ones_col")
            nc.vector.memset(ones_col[:], 1.0)
            # preload the Exp activation table during the DMA window
            warm = big.tile([1, 1], F32, tag="warm")
            nc.vector.memset(warm[:], 0.0)
            nc.scalar.activation(warm[:], warm[:], EXP)
            # ramp the PE p-state with dummy matmuls while inputs stream in
            wmm = big.tile([128, 512], F16, tag="wmm")
            nc.vector.memset(wmm[:], 0.0)
            wps = ps.tile([128, S], F32, tag="mm", name="wps")
            for _ in range(5):
                nc.tensor.matmul(wps[:], wmm[:, :128], wmm[:],
                                 start=True, stop=True)

            # scores + exp + mask for all 4 heads (et flat [128, 4*S] per head)
            ets = []
            for hh in range(4):
                hp2, p0 = hh // 2, 64 * (hh % 2)
                et = etp.tile([128, 4 * S], F16, tag="et", name=f"et{hh}")
                ets.append(et)
                for kt in range(4):
                    q0 = 128 * kt
                    sp = ps.tile([128, S], F32, tag="mm", name="sp")
                    nc.tensor.matmul(
                        sp[:, q0:S],
                        kt_[p0:p0 + 64, hp2, q0:q0 + 128],
                        qt[p0:p0 + 64, hp2, q0:S])
                    nc.scalar.activation(et[:, kt * S + q0:kt * S + S],
                                         sp[:, q0:S], EXP, scale=scale)
                    eng = nc.gpsimd if kt % 2 else nc.vector
                    eng.tensor_mul(et[:, kt * S + q0:kt * S + q0 + 128],
                                   et[:, kt * S + q0:kt * S + q0 + 128], tri[:])

            # fused (AV ; Z) per head, software-pipelined with zb broadcasts
            ops, zrs, zbs = [None] * 4, [None] * 4, [None] * 4

            def emit_av(hh):
                et = ets[hh]
                op = psb.tile([128, S], F32, tag="op", name=f"op{hh}")
                ops[hh] = op
                for kt in range(4):
                    nc.tensor.matmul(op[:65, 128 * kt:S],
                                     vt[:, kt, hh, :],
                                     et[:, kt * S + 128 * kt:kt * S + S],
                                     start=(kt == 0), stop=(kt == 3))
                zr = etp.tile([1, S], F16, tag="zr", name=f"zr{hh}")
                zrs[hh] = zr
                with nc.allow_low_precision(reason="1/Z broadcast; fp16 ample"):
                    nc.vector.reciprocal(zr[:], op[64:65, :])

            def emit_zb(hh):
                zbp = psz.tile([128, S], F32, tag="zb", name=f"zb{hh}")[:64, :]
                nc.tensor.matmul(zbp[:], ones_col[:], zrs[hh][:])
                zb = etp.tile([64, S], F32R, tag="zbs", name=f"zbs{hh}")
                nc.vector.tensor_copy(zb[:], zbp[:])
                zbs[hh] = zb

            att = big.tile([128, 2, S], F32R, tag="att")

            def emit_mul(hh):
                hp2, p0 = hh // 2, 64 * (hh % 2)
                nc.vector.tensor_mul(att[p0:p0 + 64, hp2, :],
                                     ops[hh][:64, :], zbs[hh][:])

            emit_av(0)
            emit_av(1)
            emit_zb(0)
            emit_av(2)
            emit_zb(1)
            emit_mul(0)
            emit_av(3)
            emit_zb(2)
            emit_mul(1)
            emit_zb(3)
            emit_mul(2)
            emit_mul(3)

            # Wo partial: part[mt] [128 d_out, S] = sum_ch wo[:,ch,mt].T @ att
            for mt in range(DT):
                wp = ps.tile([128, S], F32, tag="mm", name="wp")
                for ch in range(2):
                    nc.tensor.matmul(wp[:], wo[:, ch, mt * 128:(mt + 1) * 128],
                                     att[:, ch, :], start=(ch == 0), stop=(ch == 1))
                ot = outp.tile([128, S], F16, tag="ot")
                if mt % 2:
                    nc.vector.tensor_copy(ot[:], wp[:])
                else:
                    nc.scalar.copy(ot[:], wp[:])
                deng = nc.scalar if mt % 2 else nc.sync
                deng.dma_start(part_d.ap()[mt], ot[:])
    nc.compile()
    return nc


def _build_C(kv16: bool):
    """Memory module, token-sharded (128 tokens per core). Inputs:
      q    [128, 128] f32      Q.T for this core's tokens (host, fp64->fp32)
      kk   [128, NK] f32       knowledge_K.T (p=r)
      kv   [128, NT, D] f32|f16  knowledge_V tiled (p=k within tile)
    Output: mo [128, D] f32|f16  memory output rows for this core's tokens.

    Score matmul is exact fp32. Selection operates on RAW scores (order is
    scale-invariant) via an is_ge threshold at the 16th-largest value."""
    nc = bacc.Bacc("TRN2", target_bir_lowering=False, debug=False,
                   num_devices=N_CORES)
    VDT = F16 if kv16 else F32R
    ODT = F16 if kv16 else F32
    q_d = nc.dram_tensor("q", [128, 128], F32, kind="ExternalInput")
    kk_d = nc.dram_tensor("kk", [128, NK], F32, kind="ExternalInput")
    kv_d = nc.dram_tensor("kv", [128, NT, D], VDT, kind="ExternalInput")
    mo_d = nc.dram_tensor("mo", [128, D], ODT, kind="ExternalOutput")
    inv_sqrt_r = float(1.0 / np.sqrt(R))

    with tile.TileContext(nc) as tc:
        with (
            tc.tile_pool(name="big", bufs=1) as big,
            tc.tile_pool(name="work", bufs=2) as work,
            tc.tile_pool(name="ps", bufs=2, space="PSUM") as ps,
            tc.tile_pool(name="pss", bufs=1, space="PSUM") as pss,
            tc.tile_pool(name="ps1", bufs=2, space="PSUM") as ps1,
        ):
            # q + kk first (scores gate on them), then kv chunks
            q = big.tile([128, 128], F32, tag="q")
            nc.sync.dma_start(q[:], q_d.ap())
            kk = big.tile([128, NK], F32, tag="kk")
            nc.sync.dma_start(kk[:], kk_d.ap())
            kvt = []
            for nt in range(NT):
                kv = big.tile([128, D], VDT, tag=f"kv{nt}", name=f"kv{nt}")
                nc.sync.dma_start(kv[:], kv_d.ap()[:, nt, :])
                kvt.append(kv)
            # preload the Exp activation table during the DMA window
            warm = big.tile([1, 1], F32, tag="warm")
            nc.vector.memset(warm[:], 0.0)
            nc.scalar.activation(warm[:], warm[:], EXP)
            # ramp the PE p-state with dummy matmuls while inputs stream in
            wmm = big.tile([128, 512], F16, tag="wmm")
            nc.vector.memset(wmm[:], 0.0)
            wps = ps.tile([128, 512], F32, tag="mm", name="wps")
            for _ in range(9):
                nc.tensor.matmul(wps[:], wmm[:, :128], wmm[:],
                                 start=True, stop=True)

            # raw scores token-major [tok, NK] in PSUM (2 banks)
            s = pss.tile([128, NK], F32, tag="s")
            for c2 in range(2):
                nc.tensor.matmul(s[:, c2 * 512:(c2 + 1) * 512], q[:],
                                 kk[:, c2 * 512:(c2 + 1) * 512])

            # top-16: 16th-largest -> tau; exp runs on Act concurrently with
            # the match_replace/max chain on DVE
            m8a = work.tile([128, 8], F32, tag="m8a")
            m8b = work.tile([128, 8], F32, tag="m8b")
            s2 = work.tile([128, NK], F32, tag="s2")
            nbias = work.tile([128, 1], F32, tag="nbias")
            me = work.tile([128, NK], F32, tag="me")
            nc.vector.max(m8a[:], s[:])
            nc.vector.tensor_scalar_mul(nbias[:], m8a[:, 0:1], -inv_sqrt_r)
            # match_replace is modeled as writing s, so it must precede the
            # exp read; max8(s2) then runs on DVE concurrently with exp on Act
            nc.vector.match_replace(s2[:], m8a[:], s[:], NEG)
            nc.scalar.activation(me[:], s[:], EXP, scale=inv_sqrt_r, bias=nbias[:])
            nc.vector.max(m8b[:], s2[:])
            tau = m8b[:, 7:8]

            # masked exp + fused Z accumulation
            etok = work.tile([128, NK], F32, tag="etok")
            z = work.tile([128, 1], F32, tag="z")
            nc.vector.scalar_tensor_tensor(etok[:], s[:], tau, me[:],
                                           op0=mybir.AluOpType.is_ge,
                                           op1=mybir.AluOpType.mult,
                                           accum_out=z[:])
            zr = work.tile([128, 1], F32, tag="zr")
            nc.vector.reciprocal(zr[:], z[:])

            # keep the PE p-state warm across the ~5us top-k chain so the
            # transposes and WV matmuls get priced at full clock
            for _ in range(12):
                nc.tensor.matmul(wps[:], wmm[:, :128], wmm[:],
                                 start=True, stop=True)

            # transpose masked exp -> per-nt eT tiles
            idn = big.tile([128, 128], F32, tag="idn")
            make_identity(nc, idn[:])
            eTs = []
            for nt in range(NT):
                tp = ps1.tile([128, 128], F32, tag="tp", name=f"tp{nt}")
                nc.tensor.transpose(tp[:], etok[:, nt * 128:(nt + 1) * 128], idn[:])
                eT = work.tile([128, 128], VDT, tag=f"eT{nt}", name=f"eT{nt}")
                if nt % 2:
                    nc.vector.tensor_copy(eT[:], tp[:])
                else:
                    nc.scalar.copy(eT[:], tp[:])
                eTs.append(eT)

            # WV: out[tok, :] = (eT.T @ kv) * zr ; nt-outer for kv streaming,
            # halves drained independently so the store tail overlaps
            op0 = ps.tile([128, 512], F32, tag="mm", name="op0")
            op1 = ps.tile([128, 512], F32, tag="mm", name="op1")
            out = work.tile([128, D], ODT, tag="out")
            for nt in range(NT - 1):
                nc.tensor.matmul(op0[:], eTs[nt][:], kvt[nt][:, 0:512],
                                 start=(nt == 0), stop=False)
                nc.tensor.matmul(op1[:], eTs[nt][:], kvt[nt][:, 512:1024],
                                 start=(nt == 0), stop=False)
            nc.tensor.matmul(op0[:], eTs[NT - 1][:], kvt[NT - 1][:, 0:512],
                             start=False, stop=True)
            nc.scalar.activation(out[:, 0:512], op0[:], COPY, scale=zr[:])
            nc.sync.dma_start(mo_d.ap()[:, 0:512], out[:, 0:512])
            nc.tensor.matmul(op1[:], eTs[NT - 1][:], kvt[NT - 1][:, 512:1024],
                             start=False, stop=True)
            nc.vector.tensor_scalar_mul(out[:, 512:1024], op1[:], zr[:])
            nc.scalar.dma_start(mo_d.ap()[:, 512:1024], out[:, 512:1024])
    nc.compile()
    return nc


def _build_D():
    """lm_head, vocab-sharded, all fp16. Inputs:
      xf  [128, DT, T] f16         xf.T tiled (DMA'd as 4 dt-pair tiles)
      hw  [128, DT, NVC, VC] f16   head_w.T slice for this core's 4000 cols
    Output: lo [T, VSL] f16."""
    nc = bacc.Bacc("TRN2", target_bir_lowering=False, debug=False,
                   num_devices=N_CORES)
    xf_d = nc.dram_tensor("xf", [128, DT, T], F16, kind="ExternalInput")
    hw_d = nc.dram_tensor("hw", [128, DT, NVC, VC], F16, kind="ExternalInput")
    lo_d = nc.dram_tensor("lo", [T, VSL], F16, kind="ExternalOutput")
    TT = T // 128  # 8 token tiles

    with tile.TileContext(nc) as tc:
        with (
            tc.tile_pool(name="big", bufs=1) as big,
            tc.tile_pool(name="wpool", bufs=6) as wpool,
            tc.tile_pool(name="opool", bufs=6) as opool,
            tc.tile_pool(name="ps", bufs=8, space="PSUM") as ps,
        ):
            xft = []
            for i in range(DT):
                xt = big.tile([128, T], F16, tag=f"xf{i}", name=f"xf{i}")
                xft.append(xt)
            hws = {}
            # ramp the PE p-state with dummy matmuls while inputs stream in
            wmm = big.tile([128, 512], F16, tag="wmm")
            nc.vector.memset(wmm[:], 0.0)
            wps = ps.tile([128, VC], F32, tag="pp", name="wps")
            for _ in range(9):
                nc.tensor.matmul(wps[:], wmm[:, :128], wmm[:, :VC],
                                 start=True, stop=True)

            def load_hw(vc, half):
                hw = wpool.tile([128, 4, VC], F16, tag="hw",
                                name=f"hw{vc}{'ab'[half]}")
                nc.scalar.dma_start(
                    hw[:], hw_d.ap()[:, 4 * half:4 * half + 4, vc, :])
                hws[(vc, half)] = hw

            # interleave: xf0, hw0a, xf1..3, hw0b, xf4..7, hw1; then stream
            nc.sync.dma_start(xft[0][:], xf_d.ap()[:, 0, :])
            load_hw(0, 0)
            for i in range(1, 4):
                nc.sync.dma_start(xft[i][:], xf_d.ap()[:, i, :])
            load_hw(0, 1)
            for i in range(4, DT):
                nc.sync.dma_start(xft[i][:], xf_d.ap()[:, i, :])
            load_hw(1, 0)
            load_hw(1, 1)

            def drain(vc, tt, pp):
                ot = opool.tile([128, VC], F16, tag="ot")
                if tt % 2 == 0:
                    nc.scalar.copy(ot[:], pp[:])
                else:
                    nc.vector.tensor_copy(ot[:], pp[:])
                deng = (nc.sync, nc.scalar)[tt % 2]
                deng.dma_start(
                    lo_d.ap()[tt * 128:(tt + 1) * 128,
                              vc * VC:(vc + 1) * VC], ot[:])

            # vc 0: dt-major so compute starts as xf/hw tiles stream in
            pps = []
            for tt in range(TT):
                pp = ps.tile([128, VC], F32, tag="pp", name=f"pp0_{tt}")
                pps.append(pp)
            for dt in range(DT):
                hw = hws[(0, dt // 4)]
                for tt in range(TT):
                    nc.tensor.matmul(pps[tt][:],
                                     xft[dt][:, tt * 128:(tt + 1) * 128],
                                     hw[:, dt % 4, :],
                                     start=(dt == 0), stop=(dt == DT - 1))
                    if dt == DT - 1:
                        drain(0, tt, pps[tt])

            load_hw(2, 0)
            load_hw(2, 1)

            # vc 1+: tt-major — each PSUM bank is held only ~1.7us, so bank
            # recycling never lands on the PE critical path and stores spread
            for vc in range(1, NVC):
                if vc + 2 < NVC:
                    load_hw(vc + 2, 0)
                    load_hw(vc + 2, 1)
                for tt in range(TT):
                    pp = ps.tile([128, VC], F32, tag="pp", name=f"pp{vc}_{tt}")
                    for dt in range(DT):
                        nc.tensor.matmul(pp[:],
                                         xft[dt][:, tt * 128:(tt + 1) * 128],
                                         hws[(vc, dt // 4)][:, dt % 4, :],
                                         start=(dt == 0), stop=(dt == DT - 1))
                    drain(vc, tt, pp)
    nc.compile()
    return nc


_PROGS = {}


def _prog(name):
    if name not in _PROGS:
        _PROGS[name] = {"A": _build_A,
                        "C1": lambda: _build_C(False),
                        "C2": lambda: _build_C(True),
                        "D": _build_D}[name]()
    return _PROGS[name]


# ---------------------------------------------------------------- host-side math


def _ln(x, w, b):
    m = x.mean(-1, keepdims=True, dtype=np.float32)
    v = ((x - m) ** 2).mean(-1, keepdims=True, dtype=np.float32)
    return ((x - m) / np.sqrt(v + np.float32(1e-5)) * w + b).astype(np.float32)


def _softmax(x, axis=-1):
    m = x.max(axis=axis, keepdims=True)
    e = np.exp(x - m)
    return e / e.sum(axis=axis, keepdims=True)


def _nw(xn, A, Bm, Wimp, Wr):
    """SSM scan + routing -> neuron weights [B, NC] (host, fp32)."""
    u = xn @ Bm                       # [B,S,SD]
    h = np.zeros((xn.shape[0], A.shape[0]), np.float32)
    for t in range(xn.shape[1]):
        h = h @ A + u[:, t]
    h_proj = h @ Wimp.T               # [B, D]
    imp = _softmax(np.einsum('bsd,bd->bs', xn, h_proj), axis=-1)
    pref = _softmax(xn @ Wr.T, axis=-1)
    nw = np.einsum('bs,bsn->bn', imp, pref)
    return (nw / (nw.sum(-1, keepdims=True) + np.float32(1e-8))).astype(np.float32)


def _tile_dmajor(a, dtype):
    """[rows(D-like), cols] -> [128, rows//128, cols] partition-major."""
    rows, cols = a.shape
    return np.ascontiguousarray(
        a.reshape(rows // 128, 128, cols).transpose(1, 0, 2), dtype=dtype)


_run_ncores = list(range(N_CORES))


def _run(name, in_maps):
    res = run_bass_kernel_spmd(_prog(name), in_maps, core_ids=_run_ncores)
    return res.results


def kernel(**inputs) -> np.ndarray:
    inp = {k: np.asarray(v) for k, v in inputs.items()}
    ids = inp['input_ids'].astype(np.int64)
    comp_f = inp['compress_neurons'].reshape(NC, -1).astype(np.float32)
    tri16 = np.triu(np.ones((128, 128), np.float16))
    kkT = np.ascontiguousarray(inp['knowledge_K'].T, np.float32)  # [R=128, NK]
    kv32 = _tile_dmajor(inp['knowledge_V'].astype(np.float32), np.float32)
    kv16 = kv32.astype(np.float16)

    x = (inp['tok_emb'][ids] + inp['pos_emb'][None, :ids.shape[1]]).astype(np.float32)

    for l in range(L):
        # ---- circuit (program A, batch x 4-head sharded) ----
        xn = _ln(x, inp['ln1_w'][l], inp['ln1_b'][l])
        nw = _nw(xn, inp['a_A'][l], inp['a_B'][l], inp['a_imp'][l], inp['a_router'][l])
        sc = (nw @ comp_f).reshape(B, D, R)
        eq = (nw @ inp['eQ'][l].reshape(NC, -1).astype(np.float32)).reshape(B, R, D)
        ek = (nw @ inp['eK'][l].reshape(NC, -1).astype(np.float32)).reshape(B, R, D)
        ev = (nw @ inp['eV'][l].reshape(NC, -1).astype(np.float32)).reshape(B, R, D)
        h = np.matmul(xn, sc)                       # [B, S, R] low-rank tokens
        Q = np.matmul(h, eq)                        # [B, S, D]
        K = np.matmul(h, ek)
        Vv = np.matmul(h, ev)
        woT = np.ascontiguousarray(inp['o_w'][l].T, dtype=np.float32)
        in_maps = []
        for c in range(N_CORES):
            b, hg = c // 4, c % 4
            hs = slice(256 * hg, 256 * hg + 256)
            qt = Q[b].T[hs].reshape(2, 128, S).transpose(1, 0, 2)
            kt = K[b].T[hs].reshape(2, 128, S).transpose(1, 0, 2)
            vt = np.ones((128, 4, 4, 65), np.float16)
            vt[:, :, :, :64] = (Vv[b][:, hs].reshape(4, 128, 4, 64)
                                .transpose(1, 0, 2, 3).astype(np.float16))
            in_maps.append({
                "qt": np.ascontiguousarray(qt, np.float16),
                "kt": np.ascontiguousarray(kt, np.float16),
                "vt": vt,
                "wo": np.ascontiguousarray(
                    woT[hs, :].reshape(2, 128, D).transpose(1, 0, 2), np.float32),
                "tri": tri16,
            })
        res = _run("A", in_maps)
        for b in range(B):
            acc = res[4 * b]["part"].astype(np.float32)
            for c in range(4 * b + 1, 4 * b + 4):
                acc = acc + res[c]["part"].astype(np.float32)
            x[b] += acc.reshape(D, S).T

        # ---- memory (program C1/C2, token-sharded) ----
        xn = _ln(x, inp['ln2_w'][l], inp['ln2_b'][l])
        nw = _nw(xn, inp['m_A'][l], inp['m_B'][l], inp['m_imp'][l], inp['m_router'][l])
        sc = (nw @ comp_f).reshape(B, D, R)
        # Q on host in fp64 (more accurate than any fp32 summation order)
        Qm = np.matmul(xn.astype(np.float64), sc.astype(np.float64))
        Qm = Qm.astype(np.float32)                  # [B, S, R]
        kv = kv32 if l == 0 else kv16
        in_maps = []
        for c in range(N_CORES):
            bc, s0 = c // 4, 128 * (c % 4)
            in_maps.append({
                "q": np.ascontiguousarray(Qm[bc, s0:s0 + 128].T),
                "kk": kkT,
                "kv": kv,
            })
        res = _run("C1" if l == 0 else "C2", in_maps)
        for c in range(N_CORES):
            bc, s0 = c // 4, 128 * (c % 4)
            x[bc, s0:s0 + 128] += res[c]["mo"].astype(np.float32)

    # ---- lm_head (program D, vocab-sharded) ----
    xf = _ln(x, inp['lnf_w'], inp['lnf_b'])
    xfT = np.concatenate([xf[b].T for b in range(B)], axis=1)  # [D, T]
    xfT16 = _tile_dmajor(xfT, np.float16)                      # [128, DT, T]
    hwT = np.ascontiguousarray(inp['head_w'].astype(np.float32).T)  # [D, V]
    in_maps = []
    for c in range(N_CORES):
        sl = hwT[:, VSL * c:VSL * (c + 1)]                     # [D, 4000]
        tiled = sl.reshape(DT, 128, NVC, VC).transpose(1, 0, 2, 3)
        in_maps.append({"xf": xfT16,
                        "hw": np.ascontiguousarray(tiled, np.float16)})
    res = _run("D", in_maps)
    logits = np.concatenate([res[c]["lo"].astype(np.float32)
                             for c in range(N_CORES)], axis=1)
    return logits.reshape(B, S, V)


# revision 61
# speedup vs baseline: 1.7409x; 1.0411x over previous
"""Trainium2 Bass kernel for nn_DAWN_41549513621652.

Strategy (8 NeuronCores, single chip, no cross-core collectives):
  Dense matmul work (attention circuit, memory WV, lm_head) runs on device;
  scalar glue (layernorm, the 512-step SSM scan, routing softmax, neuron-pool
  contractions, partial-sum reductions) runs on host between launches.

  5 device launches per call:
    A  (x2): circuit, sharded (batch x 4-head-group) per core. fp16
             internals (validated: attention-internal quantization does not
             perturb the residual stream enough to flip the memory module's
             top-16 selection), fp32 att->Wo path and fp32 partial outputs.
    C1/C2  : memory module, token-sharded (128 tokens/core). The top-16
             selection path (Q, scores, max8 chain) is exact fp32 with the
             same matmul shapes/order as the reference-validated baseline.
             Selection uses RAW scores (scale-invariant order) with an
             is_ge-16th-value threshold. Layer 1's value path stays fp32
             (its output feeds layer 2's selection); layer 2's kV/exp run
             fp16 (only feeds the logits).
    D  (x1): lm_head, vocab-sharded 4000 cols/core, all fp16 (validated
             2.7e-4 rel err). Weights streamed and double-buffered behind
             the PE, which is the roofline engine (~107us of fp32-accum
             fp16 matmul per core).
"""

import numpy as np

import concourse.bass as bass
import concourse.bacc as bacc
import concourse.mybir as mybir
import concourse.tile as tile
from concourse.bass_utils import run_bass_kernel_spmd
from concourse.masks import make_identity

F32 = mybir.dt.float32
F32R = mybir.dt.float32r
F16 = mybir.dt.float16

# model dims (hardcoded per problem spec)
L, D, H, R, NC, NK, KK, SD, V, B, S = 2, 1024, 16, 128, 64, 1024, 16, 64, 32000, 2, 512
DH = D // H          # 64
T = B * S            # 1024
N_CORES = 8
VSL = V // N_CORES   # 4000 per-core vocab slice
NVC = 8              # vocab chunks per core
VC = VSL // NVC      # 500 cols per chunk
DT = D // 128        # 8 d-tiles
NT = NK // 128       # 8 knowledge tiles
NEG = -1e30
EXP = mybir.ActivationFunctionType.Exp
COPY = mybir.ActivationFunctionType.Copy


# ---------------------------------------------------------------- device programs


def _build_A():
    """Circuit attention; core owns ONE batch element and FOUR heads. The
    tiny low-rank h/Q/K/V expansions (~0.2 GFLOP) are host-side glue.
    Inputs (host-prearranged partition-major):
      qt/kt [128, 2, S] f16    Q.T / K.T for this core's 4 heads
      vt  [128, 4, 4, 65] f16  V token-major per (st, hh), ones col baked in
      wo  [128, 2, D] f32r     o_w.T rows for this core's 256-wide d_in slice
      tri [128, 128] f16       upper-tri (incl diag) causal mask
    Output:
      part [DT, 128, S] f16  Wo partial for this core's batch, d-major tiles
    """
    nc = bacc.Bacc("TRN2", target_bir_lowering=False, debug=False,
                   num_devices=N_CORES)
    qt_d = nc.dram_tensor("qt", [128, 2, S], F16, kind="ExternalInput")
    kt_d = nc.dram_tensor("kt", [128, 2, S], F16, kind="ExternalInput")
    vt_d = nc.dram_tensor("vt", [128, 4, 4, 65], F16, kind="ExternalInput")
    wo_d = nc.dram_tensor("wo", [128, 2, D], F32R, kind="ExternalInput")
    tri_d = nc.dram_tensor("tri", [128, 128], F16, kind="ExternalInput")
    part_d = nc.dram_tensor("part", [DT, 128, S], F16, kind="ExternalOutput")
    scale = float(1.0 / np.sqrt(DH))

    with tile.TileContext(nc) as tc:
        with (
            tc.tile_pool(name="big", bufs=1) as big,
            tc.tile_pool(name="etp", bufs=4) as etp,
            tc.tile_pool(name="ps", bufs=3, space="PSUM") as ps,
            tc.tile_pool(name="psb", bufs=2, space="PSUM") as psb,
            tc.tile_pool(name="psz", bufs=2, space="PSUM") as psz,
            tc.tile_pool(name="out", bufs=3) as outp,
        ):
            # inputs: host precomputes the h/Q/K/V low-rank expansions
            qt = big.tile([128, 2, S], F16, tag="qt")
            nc.sync.dma_start(qt[:], qt_d.ap())
            kt_ = big.tile([128, 2, S], F16, tag="kt")
            nc.sync.dma_start(kt_[:], kt_d.ap())
            vt = big.tile([128, 4, 4, 65], F16, tag="vt")
            nc.sync.dma_start(vt[:], vt_d.ap())
            tri = big.tile([128, 128], F16, tag="tri")
            nc.sync.dma_start(tri[:], tri_d.ap())
            wo = big.tile([128, 2, D], F32R, tag="wo")
            nc.sync.dma_start(wo[:], wo_d.ap())

            ones_col = big.tile([1, 64], F16, tag="ones_col")
            nc.vector.memset(ones_col[:], 1.0)
            # preload the Exp activation table during the DMA window
            warm = big.tile([1, 1], F32, tag="warm")
            nc.vector.memset(warm[:], 0.0)
            nc.scalar.activation(warm[:], warm[:], EXP)
            # ramp the PE p-state with dummy matmuls while inputs stream in
            wmm = big.tile([128, 512], F16, tag="wmm")
            nc.vector.memset(wmm[:], 0.0)
            wps = ps.tile([128, S], F32, tag="mm", name="wps")
            for _ in range(5):
                nc.tensor.matmul(wps[:], wmm[:, :128], wmm[:],
                                 start=True, stop=True)

            # scores + exp + mask for all 4 heads (et flat [128, 4*S] per head)
            ets = []
            for hh in range(4):
                hp2, p0 = hh // 2, 64 * (hh % 2)
                et = etp.tile([128, 4 * S], F16, tag="et", name=f"et{hh}")
                ets.append(et)
                for kt in range(4):
                    q0 = 128 * kt
                    sp = ps.tile([128, S], F32, tag="mm", name="sp")
                    nc.tensor.matmul(
                        sp[:, q0:S],
                        kt_[p0:p0 + 64, hp2, q0:q0 + 128],
                        qt[p0:p0 + 64, hp2, q0:S])
                    nc.scalar.activation(et[:, kt * S + q0:kt * S + S],
                                         sp[:, q0:S], EXP, scale=scale)
                    eng = nc.gpsimd if kt % 2 else nc.vector
                    eng.tensor_mul(et[:, kt * S + q0:kt * S + q0 + 128],
                                   et[:, kt * S + q0:kt * S + q0 + 128], tri[:])

            # fused (AV ; Z) per head, software-pipelined with zb broadcasts
            ops, zrs, zbs = [None] * 4, [None] * 4, [None] * 4

            def emit_av(hh):
                et = ets[hh]
                op = psb.tile([128, S], F32, tag="op", name=f"op{hh}")
                ops[hh] = op
                for kt in range(4):
                    nc.tensor.matmul(op[:65, 128 * kt:S],
                                     vt[:, kt, hh, :],
                                     et[:, kt * S + 128 * kt:kt * S + S],
                                     start=(kt == 0), stop=(kt == 3))
                zr = etp.tile([1, S], F16, tag="zr", name=f"zr{hh}")
                zrs[hh] = zr
                with nc.allow_low_precision(reason="1/Z scale; fp16 ample"):
                    nc.vector.reciprocal(zr[:], op[64:65, :])

            def emit_zb(hh):
                zb = etp.tile([64, S], F16, tag="zbs", name=f"zbs{hh}")
                nc.gpsimd.partition_broadcast(zb[:], zrs[hh][:])
                zbs[hh] = zb

            att = big.tile([128, 2, S], F32R, tag="att")

            def emit_mul(hh):
                hp2, p0 = hh // 2, 64 * (hh % 2)
                nc.vector.tensor_mul(att[p0:p0 + 64, hp2, :],
                                     ops[hh][:64, :], zbs[hh][:])

            emit_av(0)
            emit_av(1)
            emit_zb(0)
            emit_av(2)
            emit_zb(1)
            emit_mul(0)
            emit_av(3)
            emit_zb(2)
            emit_mul(1)
            emit_zb(3)
            emit_mul(2)
            emit_mul(3)

            # Wo partial: part[mt] [128 d_out, S] = sum_ch wo[:,ch,mt].T @ att;
            # early mts store in pairs, the last two store solo (small final
            # chain), and the very last copy is split across DVE+Act
            def wo_mm(mt):
                wp = ps.tile([128, S], F32, tag="mm", name="wp")
                for ch in range(2):
                    nc.tensor.matmul(wp[:], wo[:, ch, mt * 128:(mt + 1) * 128],
                                     att[:, ch, :], start=(ch == 0), stop=(ch == 1))
                return wp

            for mp in range(3):
                ot = outp.tile([128, 2, S], F16, tag="ot")
                for half in range(2):
                    wp = wo_mm(2 * mp + half)
                    if half:
                        nc.vector.tensor_copy(ot[:, half, :], wp[:])
                    else:
                        nc.scalar.copy(ot[:, half, :], wp[:])
                deng = nc.scalar if mp % 2 else nc.sync
                deng.dma_start(
                    part_d.ap()[2 * mp:2 * mp + 2].rearrange("m p s -> p m s"),
                    ot[:])
            wp6 = wo_mm(6)
            ot6 = outp.tile([128, S], F16, tag="ot6")
            nc.scalar.copy(ot6[:], wp6[:])
            nc.sync.dma_start(part_d.ap()[6], ot6[:])
            wp7 = wo_mm(7)
            ot7 = outp.tile([128, S], F16, tag="ot7")
            nc.vector.tensor_copy(ot7[:, :256], wp7[:, :256])
            nc.scalar.copy(ot7[:, 256:], wp7[:, 256:])
            nc.scalar.dma_start(part_d.ap()[7], ot7[:])
    nc.compile()
    return nc


def _build_C(kv16: bool):
    """Memory module, token-sharded (128 tokens per core). Inputs:
      q    [128, 128] f32      Q.T for this core's tokens (host, fp64->fp32)
      kk   [128, NK] f32       knowledge_K.T (p=r)
      kv   [128, NT, D] f32|f16  knowledge_V tiled (p=k within tile)
    Output: mo [128, D] f32|f16  memory output rows for this core's tokens.

    Score matmul is exact fp32. Selection operates on RAW scores (order is
    scale-invariant) via an is_ge threshold at the 16th-largest value."""
    nc = bacc.Bacc("TRN2", target_bir_lowering=False, debug=False,
                   num_devices=N_CORES)
    VDT = F16 if kv16 else F32R
    ODT = F16 if kv16 else F32
    q_d = nc.dram_tensor("q", [128, 128], F32, kind="ExternalInput")
    kk_d = nc.dram_tensor("kk", [128, NK], F32, kind="ExternalInput")
    kv_d = nc.dram_tensor("kv", [128, NT, D], VDT, kind="ExternalInput")
    mo_d = nc.dram_tensor("mo", [128, D], ODT, kind="ExternalOutput")
    inv_sqrt_r = float(1.0 / np.sqrt(R))

    with tile.TileContext(nc) as tc:
        with (
            tc.tile_pool(name="big", bufs=1) as big,
            tc.tile_pool(name="work", bufs=2) as work,
            tc.tile_pool(name="ps", bufs=2, space="PSUM") as ps,
            tc.tile_pool(name="pss", bufs=1, space="PSUM") as pss,
            tc.tile_pool(name="ps1", bufs=2, space="PSUM") as ps1,
        ):
            # q + kk first (scores gate on them), then kv chunks
            q = big.tile([128, 128], F32, tag="q")
            nc.sync.dma_start(q[:], q_d.ap())
            kk = big.tile([128, NK], F32, tag="kk")
            nc.sync.dma_start(kk[:], kk_d.ap())
            kvt = []
            for nt in range(NT):
                kv = big.tile([128, D], VDT, tag=f"kv{nt}", name=f"kv{nt}")
                nc.sync.dma_start(kv[:], kv_d.ap()[:, nt, :])
                kvt.append(kv)
            # preload the Exp activation table during the DMA window
            warm = big.tile([1, 1], F32, tag="warm")
            nc.vector.memset(warm[:], 0.0)
            nc.scalar.activation(warm[:], warm[:], EXP)
            # ramp the PE p-state with dummy matmuls while inputs stream in
            wmm = big.tile([128, 512], F16, tag="wmm")
            nc.vector.memset(wmm[:], 0.0)
            wps = ps.tile([128, 512], F32, tag="mm", name="wps")
            for _ in range(9):
                nc.tensor.matmul(wps[:], wmm[:, :128], wmm[:],
                                 start=True, stop=True)

            # raw scores token-major [tok, NK] in PSUM (2 banks)
            s = pss.tile([128, NK], F32, tag="s")
            for c2 in range(2):
                nc.tensor.matmul(s[:, c2 * 512:(c2 + 1) * 512], q[:],
                                 kk[:, c2 * 512:(c2 + 1) * 512])

            # top-16: 16th-largest -> tau; exp runs on Act concurrently with
            # the match_replace/max chain on DVE
            m8a = work.tile([128, 8], F32, tag="m8a")
            m8b = work.tile([128, 8], F32, tag="m8b")
            s2 = work.tile([128, NK], F32, tag="s2")
            nbias = work.tile([128, 1], F32, tag="nbias")
            me = work.tile([128, NK], F32, tag="me")
            nc.vector.max(m8a[:], s[:])
            nc.vector.tensor_scalar_mul(nbias[:], m8a[:, 0:1], -inv_sqrt_r)
            # match_replace is modeled as writing s, so it must precede the
            # exp read; max8(s2) then runs on DVE concurrently with exp on Act
            nc.vector.match_replace(s2[:], m8a[:], s[:], NEG)
            nc.scalar.activation(me[:], s[:], EXP, scale=inv_sqrt_r, bias=nbias[:])
            nc.vector.max(m8b[:], s2[:])
            tau = m8b[:, 7:8]

            # masked exp + fused Z accumulation
            etok = work.tile([128, NK], F32, tag="etok")
            z = work.tile([128, 1], F32, tag="z")
            nc.vector.scalar_tensor_tensor(etok[:], s[:], tau, me[:],
                                           op0=mybir.AluOpType.is_ge,
                                           op1=mybir.AluOpType.mult,
                                           accum_out=z[:])
            zr = work.tile([128, 1], F32, tag="zr")
            nc.vector.reciprocal(zr[:], z[:])

            # keep the PE p-state warm across the ~5us top-k chain so the
            # transposes and WV matmuls get priced at full clock
            for _ in range(20):
                nc.tensor.matmul(wps[:], wmm[:, :128], wmm[:],
                                 start=True, stop=True)

            # transpose masked exp -> per-nt eT tiles
            idn = big.tile([128, 128], F32, tag="idn")
            make_identity(nc, idn[:])
            eTs = []
            for nt in range(NT):
                tp = ps1.tile([128, 128], F32, tag="tp", name=f"tp{nt}")
                nc.tensor.transpose(tp[:], etok[:, nt * 128:(nt + 1) * 128], idn[:])
                eT = work.tile([128, 128], VDT, tag=f"eT{nt}", name=f"eT{nt}")
                if nt % 2:
                    nc.vector.tensor_copy(eT[:], tp[:])
                else:
                    nc.scalar.copy(eT[:], tp[:])
                eTs.append(eT)

            # WV: out[tok, :] = (eT.T @ kv) * zr ; nt-outer for kv streaming,
            # halves drained independently so the store tail overlaps
            op0 = ps.tile([128, 512], F32, tag="mm", name="op0")
            op1 = ps.tile([128, 512], F32, tag="mm", name="op1")
            out = work.tile([128, D], ODT, tag="out")
            for nt in range(NT - 1):
                nc.tensor.matmul(op0[:], eTs[nt][:], kvt[nt][:, 0:512],
                                 start=(nt == 0), stop=False)
                nc.tensor.matmul(op1[:], eTs[nt][:], kvt[nt][:, 512:1024],
                                 start=(nt == 0), stop=False)
            nc.tensor.matmul(op0[:], eTs[NT - 1][:], kvt[NT - 1][:, 0:512],
                             start=False, stop=True)
            nc.scalar.activation(out[:, 0:512], op0[:], COPY, scale=zr[:])
            nc.sync.dma_start(mo_d.ap()[:, 0:512], out[:, 0:512])
            nc.tensor.matmul(op1[:], eTs[NT - 1][:], kvt[NT - 1][:, 512:1024],
                             start=False, stop=True)
            nc.vector.tensor_scalar_mul(out[:, 512:1024], op1[:], zr[:])
            nc.scalar.dma_start(mo_d.ap()[:, 512:1024], out[:, 512:1024])
    nc.compile()
    return nc


def _build_D():
    """lm_head, vocab-sharded, all fp16. Inputs:
      xf  [128, DT, T] f16         xf.T tiled (DMA'd as 4 dt-pair tiles)
      hw  [128, DT, NVC, VC] f16   head_w.T slice for this core's 4000 cols
    Output: lo [T, VSL] f16."""
    nc = bacc.Bacc("TRN2", target_bir_lowering=False, debug=False,
                   num_devices=N_CORES)
    xf_d = nc.dram_tensor("xf", [128, DT, T], F16, kind="ExternalInput")
    hw_d = nc.dram_tensor("hw", [128, DT, NVC, VC], F16, kind="ExternalInput")
    lo_d = nc.dram_tensor("lo", [T, VSL], F16, kind="ExternalOutput")
    TT = T // 128  # 8 token tiles

    with tile.TileContext(nc) as tc:
        with (
            tc.tile_pool(name="big", bufs=1) as big,
            tc.tile_pool(name="wpool", bufs=6) as wpool,
            tc.tile_pool(name="opool", bufs=6) as opool,
            tc.tile_pool(name="ps", bufs=8, space="PSUM") as ps,
        ):
            xft = []
            for i in range(DT):
                xt = big.tile([128, T], F16, tag=f"xf{i}", name=f"xf{i}")
                xft.append(xt)
            hws = {}
            # ramp the PE p-state with dummy matmuls while inputs stream in
            wmm = big.tile([128, 512], F16, tag="wmm")
            nc.vector.memset(wmm[:], 0.0)
            wps = ps.tile([128, VC], F32, tag="pp", name="wps")
            for _ in range(9):
                nc.tensor.matmul(wps[:], wmm[:, :128], wmm[:, :VC],
                                 start=True, stop=True)

            def load_hw(vc, half):
                hw = wpool.tile([128, 4, VC], F16, tag="hw",
                                name=f"hw{vc}{'ab'[half]}")
                nc.scalar.dma_start(
                    hw[:], hw_d.ap()[:, 4 * half:4 * half + 4, vc, :])
                hws[(vc, half)] = hw

            # interleave: xf0, hw0a, xf1..3, hw0b, xf4..7, hw1; then stream
            nc.sync.dma_start(xft[0][:], xf_d.ap()[:, 0, :])
            load_hw(0, 0)
            for i in range(1, 4):
                nc.sync.dma_start(xft[i][:], xf_d.ap()[:, i, :])
            load_hw(0, 1)
            for i in range(4, DT):
                nc.sync.dma_start(xft[i][:], xf_d.ap()[:, i, :])
            load_hw(1, 0)
            load_hw(1, 1)

            def drain(vc, tt, pp):
                ot = opool.tile([128, VC], F16, tag="ot")
                if tt % 2 == 0:
                    nc.scalar.copy(ot[:], pp[:])
                else:
                    nc.vector.tensor_copy(ot[:], pp[:])
                deng = (nc.sync, nc.scalar)[tt % 2]
                deng.dma_start(
                    lo_d.ap()[tt * 128:(tt + 1) * 128,
                              vc * VC:(vc + 1) * VC], ot[:])

            # vc 0: dt-major so compute starts as xf/hw tiles stream in
            pps = []
            for tt in range(TT):
                pp = ps.tile([128, VC], F32, tag="pp", name=f"pp0_{tt}")
                pps.append(pp)
            for dt in range(DT):
                hw = hws[(0, dt // 4)]
                for tt in range(TT):
                    nc.tensor.matmul(pps[tt][:],
                                     xft[dt][:, tt * 128:(tt + 1) * 128],
                                     hw[:, dt % 4, :],
                                     start=(dt == 0), stop=(dt == DT - 1))
                    if dt == DT - 1:
                        drain(0, tt, pps[tt])

            load_hw(2, 0)
            load_hw(2, 1)

            # vc 1+: tt-major — each PSUM bank is held only ~1.7us, so bank
            # recycling never lands on the PE critical path and stores spread
            for vc in range(1, NVC):
                if vc + 2 < NVC:
                    load_hw(vc + 2, 0)
                    load_hw(vc + 2, 1)
                for tt in range(TT):
                    pp = ps.tile([128, VC], F32, tag="pp", name=f"pp{vc}_{tt}")
                    for dt in range(DT):
                        nc.tensor.matmul(pp[:],
                                         xft[dt][:, tt * 128:(tt + 1) * 128],
                                         hws[(vc, dt // 4)][:, dt % 4, :],
                                         start=(dt == 0), stop=(dt == DT - 1))
                    drain(vc, tt, pp)
    nc.compile()
    return nc


_PROGS = {}


def _prog(name):
    if name not in _PROGS:
        _PROGS[name] = {"A": _build_A,
                        "C1": lambda: _build_C(False),
                        "C2": lambda: _build_C(True),
                        "D": _build_D}[name]()
    return _PROGS[name]


# ---------------------------------------------------------------- host-side math


def _ln(x, w, b):
    m = x.mean(-1, keepdims=True, dtype=np.float32)
    v = ((x - m) ** 2).mean(-1, keepdims=True, dtype=np.float32)
    return ((x - m) / np.sqrt(v + np.float32(1e-5)) * w + b).astype(np.float32)


def _softmax(x, axis=-1):
    m = x.max(axis=axis, keepdims=True)
    e = np.exp(x - m)
    return e / e.sum(axis=axis, keepdims=True)


def _nw(xn, A, Bm, Wimp, Wr):
    """SSM scan + routing -> neuron weights [B, NC] (host, fp32)."""
    u = xn @ Bm                       # [B,S,SD]
    h = np.zeros((xn.shape[0], A.shape[0]), np.float32)
    for t in range(xn.shape[1]):
        h = h @ A + u[:, t]
    h_proj = h @ Wimp.T               # [B, D]
    imp = _softmax(np.einsum('bsd,bd->bs', xn, h_proj), axis=-1)
    pref = _softmax(xn @ Wr.T, axis=-1)
    nw = np.einsum('bs,bsn->bn', imp, pref)
    return (nw / (nw.sum(-1, keepdims=True) + np.float32(1e-8))).astype(np.float32)


def _tile_dmajor(a, dtype):
    """[rows(D-like), cols] -> [128, rows//128, cols] partition-major."""
    rows, cols = a.shape
    return np.ascontiguousarray(
        a.reshape(rows // 128, 128, cols).transpose(1, 0, 2), dtype=dtype)


_run_ncores = list(range(N_CORES))


def _run(name, in_maps):
    res = run_bass_kernel_spmd(_prog(name), in_maps, core_ids=_run_ncores)
    return res.results


def kernel(**inputs) -> np.ndarray:
    inp = {k: np.asarray(v) for k, v in inputs.items()}
    ids = inp['input_ids'].astype(np.int64)
    comp_f = inp['compress_neurons'].reshape(NC, -1).astype(np.float32)
    tri16 = np.triu(np.ones((128, 128), np.float16))
    kkT = np.ascontiguousarray(inp['knowledge_K'].T, np.float32)  # [R=128, NK]
    kv32 = _tile_dmajor(inp['knowledge_V'].astype(np.float32), np.float32)
    kv16 = kv32.astype(np.float16)

    x = (inp['tok_emb'][ids] + inp['pos_emb'][None, :ids.shape[1]]).astype(np.float32)

    for l in range(L):
        # ---- circuit (program A, batch x 4-head sharded) ----
        xn = _ln(x, inp['ln1_w'][l], inp['ln1_b'][l])
        nw = _nw(xn, inp['a_A'][l], inp['a_B'][l], inp['a_imp'][l], inp['a_router'][l])
        sc = (nw @ comp_f).reshape(B, D, R)
        eq = (nw @ inp['eQ'][l].reshape(NC, -1).astype(np.float32)).reshape(B, R, D)
        ek = (nw @ inp['eK'][l].reshape(NC, -1).astype(np.float32)).reshape(B, R, D)
        ev = (nw @ inp['eV'][l].reshape(NC, -1).astype(np.float32)).reshape(B, R, D)
        h = np.matmul(xn, sc)                       # [B, S, R] low-rank tokens
        Q = np.matmul(h, eq)                        # [B, S, D]
        K = np.matmul(h, ek)
        Vv = np.matmul(h, ev)
        woT = np.ascontiguousarray(inp['o_w'][l].T, dtype=np.float32)
        in_maps = []
        for c in range(N_CORES):
            b, hg = c // 4, c % 4
            hs = slice(256 * hg, 256 * hg + 256)
            qt = Q[b].T[hs].reshape(2, 128, S).transpose(1, 0, 2)
            kt = K[b].T[hs].reshape(2, 128, S).transpose(1, 0, 2)
            vt = np.ones((128, 4, 4, 65), np.float16)
            vt[:, :, :, :64] = (Vv[b][:, hs].reshape(4, 128, 4, 64)
                                .transpose(1, 0, 2, 3).astype(np.float16))
            in_maps.append({
                "qt": np.ascontiguousarray(qt, np.float16),
                "kt": np.ascontiguousarray(kt, np.float16),
                "vt": vt,
                "wo": np.ascontiguousarray(
                    woT[hs, :].reshape(2, 128, D).transpose(1, 0, 2), np.float32),
                "tri": tri16,
            })
        res = _run("A", in_maps)
        for b in range(B):
            acc = res[4 * b]["part"].astype(np.float32)
            for c in range(4 * b + 1, 4 * b + 4):
                acc = acc + res[c]["part"].astype(np.float32)
            x[b] += acc.reshape(D, S).T

        # ---- memory (program C1/C2, token-sharded) ----
        xn = _ln(x, inp['ln2_w'][l], inp['ln2_b'][l])
        nw = _nw(xn, inp['m_A'][l], inp['m_B'][l], inp['m_imp'][l], inp['m_router'][l])
        sc = (nw @ comp_f).reshape(B, D, R)
        # Q on host in fp64 (more accurate than any fp32 summation order)
        Qm = np.matmul(xn.astype(np.float64), sc.astype(np.float64))
        Qm = Qm.astype(np.float32)                  # [B, S, R]
        kv = kv32 if l == 0 else kv16
        in_maps = []
        for c in range(N_CORES):
            bc, s0 = c // 4, 128 * (c % 4)
            in_maps.append({
                "q": np.ascontiguousarray(Qm[bc, s0:s0 + 128].T),
                "kk": kkT,
                "kv": kv,
            })
        res = _run("C1" if l == 0 else "C2", in_maps)
        for c in range(N_CORES):
            bc, s0 = c // 4, 128 * (c % 4)
            x[bc, s0:s0 + 128] += res[c]["mo"].astype(np.float32)

    # ---- lm_head (program D, vocab-sharded) ----
    xf = _ln(x, inp['lnf_w'], inp['lnf_b'])
    xfT = np.concatenate([xf[b].T for b in range(B)], axis=1)  # [D, T]
    xfT16 = _tile_dmajor(xfT, np.float16)                      # [128, DT, T]
    hwT = np.ascontiguousarray(inp['head_w'].astype(np.float32).T)  # [D, V]
    in_maps = []
    for c in range(N_CORES):
        sl = hwT[:, VSL * c:VSL * (c + 1)]                     # [D, 4000]
        tiled = sl.reshape(DT, 128, NVC, VC).transpose(1, 0, 2, 3)
        in_maps.append({"xf": xfT16,
                        "hw": np.ascontiguousarray(tiled, np.float16)})
    res = _run("D", in_maps)
    logits = np.concatenate([res[c]["lo"].astype(np.float32)
                             for c in range(N_CORES)], axis=1)
    return logits.reshape(B, S, V)


# revision 74
# speedup vs baseline: 1.7559x; 1.0086x over previous
"""Trainium2 Bass kernel for nn_DAWN_41549513621652.

Strategy (8 NeuronCores, single chip, no cross-core collectives):
  Dense matmul work (attention circuit, memory WV, lm_head) runs on device;
  scalar glue (layernorm, the 512-step SSM scan, routing softmax, neuron-pool
  contractions, partial-sum reductions) runs on host between launches.

  5 device launches per call:
    A  (x2): circuit, sharded (batch x 4-head-group) per core. fp16
             internals (validated: attention-internal quantization does not
             perturb the residual stream enough to flip the memory module's
             top-16 selection), fp32 att->Wo path and fp32 partial outputs.
    C1/C2  : memory module, token-sharded (128 tokens/core). The top-16
             selection path (Q, scores, max8 chain) is exact fp32 with the
             same matmul shapes/order as the reference-validated baseline.
             Selection uses RAW scores (scale-invariant order) with an
             is_ge-16th-value threshold. Layer 1's value path stays fp32
             (its output feeds layer 2's selection); layer 2's kV/exp run
             fp16 (only feeds the logits).
    D  (x1): lm_head, vocab-sharded 4000 cols/core, all fp16 (validated
             2.7e-4 rel err). Weights streamed and double-buffered behind
             the PE, which is the roofline engine (~107us of fp32-accum
             fp16 matmul per core).
"""

import numpy as np

import concourse.bass as bass
import concourse.bacc as bacc
import concourse.mybir as mybir
import concourse.tile as tile
from concourse.bass_utils import run_bass_kernel_spmd
from concourse.masks import make_identity

F32 = mybir.dt.float32
F32R = mybir.dt.float32r
F16 = mybir.dt.float16

# model dims (hardcoded per problem spec)
L, D, H, R, NC, NK, KK, SD, V, B, S = 2, 1024, 16, 128, 64, 1024, 16, 64, 32000, 2, 512
DH = D // H          # 64
T = B * S            # 1024
N_CORES = 8
VSL = V // N_CORES   # 4000 per-core vocab slice
NVC = 8              # vocab chunks per core
VC = VSL // NVC      # 500 cols per chunk
DT = D // 128        # 8 d-tiles
NT = NK // 128       # 8 knowledge tiles
NEG = -1e30
EXP = mybir.ActivationFunctionType.Exp
COPY = mybir.ActivationFunctionType.Copy


# ---------------------------------------------------------------- device programs


def _build_A():
    """Circuit attention; core owns ONE batch element and FOUR heads. The
    tiny low-rank h/Q/K/V expansions (~0.2 GFLOP) are host-side glue.
    Inputs (host-prearranged partition-major):
      qt/kt [128, 2, S] f16    Q.T / K.T for this core's 4 heads
      vt  [128, 4, 4, 65] f16  V token-major per (st, hh), ones col baked in
      wo  [128, 2, D] f32r     o_w.T rows for this core's 256-wide d_in slice
      tri [128, 128] f16       upper-tri (incl diag) causal mask
    Output:
      part [DT, 128, S] f16  Wo partial for this core's batch, d-major tiles
    """
    nc = bacc.Bacc("TRN2", target_bir_lowering=False, debug=False,
                   num_devices=N_CORES)
    qt_d = nc.dram_tensor("qt", [128, 2, S], F16, kind="ExternalInput")
    kt_d = nc.dram_tensor("kt", [128, 2, S], F16, kind="ExternalInput")
    vt_d = nc.dram_tensor("vt", [128, 4, 4, 65], F16, kind="ExternalInput")
    wo_d = nc.dram_tensor("wo", [128, 2, D], F32R, kind="ExternalInput")
    tri_d = nc.dram_tensor("tri", [128, 128], F16, kind="ExternalInput")
    part_d = nc.dram_tensor("part", [DT, 128, S], F16, kind="ExternalOutput")
    scale = float(1.0 / np.sqrt(DH))

    with tile.TileContext(nc) as tc:
        with (
            tc.tile_pool(name="big", bufs=1) as big,
            tc.tile_pool(name="etp", bufs=4) as etp,
            tc.tile_pool(name="ps", bufs=3, space="PSUM") as ps,
            tc.tile_pool(name="psb", bufs=2, space="PSUM") as psb,
            tc.tile_pool(name="psz", bufs=2, space="PSUM") as psz,
            tc.tile_pool(name="out", bufs=3) as outp,
        ):
            # inputs: host precomputes the h/Q/K/V low-rank expansions
            qts, kts = [], []
            for hp2 in range(2):
                qh = big.tile([128, S], F16, tag=f"qt{hp2}", name=f"qt{hp2}")
                kh = big.tile([128, S], F16, tag=f"kt{hp2}", name=f"kt{hp2}")
                nc.sync.dma_start(qh[:], qt_d.ap()[:, hp2, :])
                nc.sync.dma_start(kh[:], kt_d.ap()[:, hp2, :])
                qts.append(qh)
                kts.append(kh)
            vt = big.tile([128, 4, 4, 65], F16, tag="vt")
            nc.sync.dma_start(vt[:], vt_d.ap())
            tri = big.tile([128, 128], F16, tag="tri")
            nc.sync.dma_start(tri[:], tri_d.ap())
            wo = big.tile([128, 2, D], F32R, tag="wo")
            nc.sync.dma_start(wo[:], wo_d.ap())

            ones_col = big.tile([1, 64], F16, tag="ones_col")
            nc.vector.memset(ones_col[:], 1.0)
            # preload the Exp activation table during the DMA window
            warm = big.tile([1, 1], F32, tag="warm")
            nc.vector.memset(warm[:], 0.0)
            nc.scalar.activation(warm[:], warm[:], EXP)
            # ramp the PE p-state with dummy matmuls while inputs stream in
            wmm = big.tile([128, 512], F16, tag="wmm")
            nc.vector.memset(wmm[:], 0.0)
            wps = ps.tile([128, S], F32, tag="mm", name="wps")
            for _ in range(5):
                nc.tensor.matmul(wps[:], wmm[:, :128], wmm[:],
                                 start=True, stop=True)

            # scores + exp + mask for all 4 heads (et flat [128, 4*S] per head)
            ets = []
            for hh in range(4):
                hp2, p0 = hh // 2, 64 * (hh % 2)
                et = etp.tile([128, 4 * S], F16, tag="et", name=f"et{hh}")
                ets.append(et)
                for kt in range(4):
                    q0 = 128 * kt
                    sp = ps.tile([128, S], F32, tag="mm", name="sp")
                    nc.tensor.matmul(
                        sp[:, q0:S],
                        kts[hp2][p0:p0 + 64, q0:q0 + 128],
                        qts[hp2][p0:p0 + 64, q0:S])
                    nc.scalar.activation(et[:, kt * S + q0:kt * S + S],
                                         sp[:, q0:S], EXP, scale=scale)
                    eng = nc.gpsimd if kt % 2 else nc.vector
                    eng.tensor_mul(et[:, kt * S + q0:kt * S + q0 + 128],
                                   et[:, kt * S + q0:kt * S + q0 + 128], tri[:])

            # fused (AV ; Z) per head, software-pipelined with zb broadcasts
            ops, zrs, zbs = [None] * 4, [None] * 4, [None] * 4

            def emit_av(hh):
                et = ets[hh]
                op = psb.tile([128, S], F32, tag="op", name=f"op{hh}")
                ops[hh] = op
                for kt in range(4):
                    nc.tensor.matmul(op[:65, 128 * kt:S],
                                     vt[:, kt, hh, :],
                                     et[:, kt * S + 128 * kt:kt * S + S],
                                     start=(kt == 0), stop=(kt == 3))
                zr = etp.tile([1, S], F16, tag="zr", name=f"zr{hh}")
                zrs[hh] = zr
                with nc.allow_low_precision(reason="1/Z scale; fp16 ample"):
                    nc.vector.reciprocal(zr[:], op[64:65, :])

            def emit_zb(hh):
                zb = etp.tile([64, S], F16, tag="zbs", name=f"zbs{hh}")
                nc.gpsimd.partition_broadcast(zb[:], zrs[hh][:])
                zbs[hh] = zb

            att = big.tile([128, 2, S], F32R, tag="att")

            def emit_mul(hh):
                hp2, p0 = hh // 2, 64 * (hh % 2)
                nc.vector.tensor_mul(att[p0:p0 + 64, hp2, :],
                                     ops[hh][:64, :], zbs[hh][:])

            emit_av(0)
            emit_av(1)
            emit_zb(0)
            emit_av(2)
            emit_zb(1)
            emit_mul(0)
            emit_av(3)
            emit_zb(2)
            emit_mul(1)
            emit_zb(3)
            emit_mul(2)
            emit_mul(3)

            # Wo partial: part[mt] [128 d_out, S] = sum_ch wo[:,ch,mt].T @ att;
            # early mts store in pairs, the last two store solo (small final
            # chain), and the very last copy is split across DVE+Act
            def wo_mm(mt):
                wp = ps.tile([128, S], F32, tag="mm", name="wp")
                for ch in range(2):
                    nc.tensor.matmul(wp[:], wo[:, ch, mt * 128:(mt + 1) * 128],
                                     att[:, ch, :], start=(ch == 0), stop=(ch == 1))
                return wp

            for mp in range(3):
                ot = outp.tile([128, 2, S], F16, tag="ot")
                for half in range(2):
                    wp = wo_mm(2 * mp + half)
                    if half:
                        nc.vector.tensor_copy(ot[:, half, :], wp[:])
                    else:
                        nc.scalar.copy(ot[:, half, :], wp[:])
                deng = nc.scalar if mp % 2 else nc.sync
                deng.dma_start(
                    part_d.ap()[2 * mp:2 * mp + 2].rearrange("m p s -> p m s"),
                    ot[:])
            wp6 = wo_mm(6)
            ot6 = outp.tile([128, S], F16, tag="ot6")
            nc.vector.tensor_copy(ot6[:], wp6[:])
            nc.sync.dma_start(part_d.ap()[6], ot6[:])
            wp7 = wo_mm(7)
            ot7 = outp.tile([128, S], F16, tag="ot7")
            nc.scalar.copy(ot7[:, :256], wp7[:, :256])
            nc.vector.tensor_copy(ot7[:, 256:], wp7[:, 256:])
            nc.scalar.dma_start(part_d.ap()[7], ot7[:])
    nc.compile()
    return nc


def _build_C(kv16: bool):
    """Memory module, token-sharded (128 tokens per core). Inputs:
      q    [128, 128] f32      Q.T for this core's tokens (host, fp64->fp32)
      kk   [128, NK] f32       knowledge_K.T (p=r)
      kv   [128, NT, D] f32|f16  knowledge_V tiled (p=k within tile)
    Output: mo [128, D] f32|f16  memory output rows for this core's tokens.

    Score matmul is exact fp32. Selection operates on RAW scores (order is
    scale-invariant) via an is_ge threshold at the 16th-largest value."""
    nc = bacc.Bacc("TRN2", target_bir_lowering=False, debug=False,
                   num_devices=N_CORES)
    VDT = F16 if kv16 else F32R
    ODT = F16 if kv16 else F32
    q_d = nc.dram_tensor("q", [128, 128], F32, kind="ExternalInput")
    kk_d = nc.dram_tensor("kk", [128, NK], F32, kind="ExternalInput")
    kv_d = nc.dram_tensor("kv", [128, NT, D], VDT, kind="ExternalInput")
    mo_d = nc.dram_tensor("mo", [128, D], ODT, kind="ExternalOutput")
    inv_sqrt_r = float(1.0 / np.sqrt(R))

    with tile.TileContext(nc) as tc:
        with (
            tc.tile_pool(name="big", bufs=1) as big,
            tc.tile_pool(name="work", bufs=2) as work,
            tc.tile_pool(name="ps", bufs=2, space="PSUM") as ps,
            tc.tile_pool(name="pss", bufs=1, space="PSUM") as pss,
            tc.tile_pool(name="ps1", bufs=2, space="PSUM") as ps1,
        ):
            # q + kk first (scores gate on them), then kv chunks
            q = big.tile([128, 128], F32, tag="q")
            nc.sync.dma_start(q[:], q_d.ap())
            kk = big.tile([128, NK], F32, tag="kk")
            nc.sync.dma_start(kk[:], kk_d.ap())
            kvt = []
            for nt in range(NT):
                kv = big.tile([128, D], VDT, tag=f"kv{nt}", name=f"kv{nt}")
                nc.sync.dma_start(kv[:], kv_d.ap()[:, nt, :])
                kvt.append(kv)
            # preload the Exp activation table during the DMA window
            warm = big.tile([1, 1], F32, tag="warm")
            nc.vector.memset(warm[:], 0.0)
            nc.scalar.activation(warm[:], warm[:], EXP)
            # ramp the PE p-state with dummy matmuls while inputs stream in
            wmm = big.tile([128, 512], F16, tag="wmm")
            nc.vector.memset(wmm[:], 0.0)
            wps = ps.tile([128, 512], F32, tag="mm", name="wps")
            for _ in range(9):
                nc.tensor.matmul(wps[:], wmm[:, :128], wmm[:],
                                 start=True, stop=True)

            # raw scores token-major [tok, NK] in PSUM (2 banks)
            s = pss.tile([128, NK], F32, tag="s")
            for c2 in range(2):
                nc.tensor.matmul(s[:, c2 * 512:(c2 + 1) * 512], q[:],
                                 kk[:, c2 * 512:(c2 + 1) * 512])

            # top-16: 16th-largest -> tau; exp runs on Act concurrently with
            # the match_replace/max chain on DVE
            m8a = work.tile([128, 8], F32, tag="m8a")
            m8b = work.tile([128, 8], F32, tag="m8b")
            s2 = work.tile([128, NK], F32, tag="s2")
            nbias = work.tile([128, 1], F32, tag="nbias")
            me = work.tile([128, NK], F32, tag="me")
            nc.vector.max(m8a[:], s[:])
            nc.vector.tensor_scalar_mul(nbias[:], m8a[:, 0:1], -inv_sqrt_r)
            # match_replace is modeled as writing s, so it must precede the
            # exp read; max8(s2) then runs on DVE concurrently with exp on Act
            nc.vector.match_replace(s2[:], m8a[:], s[:], NEG)
            nc.scalar.activation(me[:], s[:], EXP, scale=inv_sqrt_r, bias=nbias[:])
            nc.vector.max(m8b[:], s2[:])
            tau = m8b[:, 7:8]

            # masked exp + fused Z accumulation
            etok = work.tile([128, NK], F32, tag="etok")
            z = work.tile([128, 1], F32, tag="z")
            nc.vector.scalar_tensor_tensor(etok[:], s[:], tau, me[:],
                                           op0=mybir.AluOpType.is_ge,
                                           op1=mybir.AluOpType.mult,
                                           accum_out=z[:])
            zr = work.tile([128, 1], F32, tag="zr")
            nc.vector.reciprocal(zr[:], z[:])

            # keep the PE p-state warm across the top-k chain (and, for the
            # fp32-kv variant, the longer kv DMA wait) so the transposes and
            # WV matmuls get priced at full clock
            for _ in range(20):
                nc.tensor.matmul(wps[:], wmm[:, :128], wmm[:],
                                 start=True, stop=True)

            # transpose masked exp -> per-nt eT tiles
            idn = big.tile([128, 128], F32, tag="idn")
            make_identity(nc, idn[:])
            eTs = []
            for nt in range(NT):
                tp = ps1.tile([128, 128], F32, tag="tp", name=f"tp{nt}")
                nc.tensor.transpose(tp[:], etok[:, nt * 128:(nt + 1) * 128], idn[:])
                eT = work.tile([128, 128], VDT, tag=f"eT{nt}", name=f"eT{nt}")
                if nt % 2:
                    nc.vector.tensor_copy(eT[:], tp[:])
                else:
                    nc.scalar.copy(eT[:], tp[:])
                eTs.append(eT)

            # WV: out[tok, :] = (eT.T @ kv) * zr ; nt-outer for kv streaming,
            # halves drained independently so the store tail overlaps
            op0 = ps.tile([128, 512], F32, tag="mm", name="op0")
            op1 = ps.tile([128, 512], F32, tag="mm", name="op1")
            out = work.tile([128, D], ODT, tag="out")
            for nt in range(NT - 1):
                nc.tensor.matmul(op0[:], eTs[nt][:], kvt[nt][:, 0:512],
                                 start=(nt == 0), stop=False)
                nc.tensor.matmul(op1[:], eTs[nt][:], kvt[nt][:, 512:1024],
                                 start=(nt == 0), stop=False)
            nc.tensor.matmul(op0[:], eTs[NT - 1][:], kvt[NT - 1][:, 0:512],
                             start=False, stop=True)
            nc.scalar.activation(out[:, 0:512], op0[:], COPY, scale=zr[:])
            nc.sync.dma_start(mo_d.ap()[:, 0:512], out[:, 0:512])
            nc.tensor.matmul(op1[:], eTs[NT - 1][:], kvt[NT - 1][:, 512:1024],
                             start=False, stop=True)
            nc.vector.tensor_scalar_mul(out[:, 512:1024], op1[:], zr[:])
            nc.scalar.dma_start(mo_d.ap()[:, 512:1024], out[:, 512:1024])
    nc.compile()
    return nc


def _build_D():
    """lm_head, vocab-sharded, all fp16. Inputs:
      xf  [128, DT, T] f16         xf.T tiled (DMA'd as 4 dt-pair tiles)
      hw  [128, DT, NVC, VC] f16   head_w.T slice for this core's 4000 cols
    Output: lo [T, VSL] f16."""
    nc = bacc.Bacc("TRN2", target_bir_lowering=False, debug=False,
                   num_devices=N_CORES)
    xf_d = nc.dram_tensor("xf", [128, DT, T], F16, kind="ExternalInput")
    hw_d = nc.dram_tensor("hw", [128, DT, NVC, VC], F16, kind="ExternalInput")
    lo_d = nc.dram_tensor("lo", [T, VSL], F16, kind="ExternalOutput")
    TT = T // 128  # 8 token tiles

    with tile.TileContext(nc) as tc:
        with (
            tc.tile_pool(name="big", bufs=1) as big,
            tc.tile_pool(name="wpool", bufs=6) as wpool,
            tc.tile_pool(name="opool", bufs=6) as opool,
            tc.tile_pool(name="ps", bufs=8, space="PSUM") as ps,
        ):
            xft = []
            for i in range(DT):
                xt = big.tile([128, T], F16, tag=f"xf{i}", name=f"xf{i}")
                xft.append(xt)
            hws = {}
            # ramp the PE p-state with dummy matmuls while inputs stream in
            wmm = big.tile([128, 512], F16, tag="wmm")
            nc.vector.memset(wmm[:], 0.0)
            wps = ps.tile([128, VC], F32, tag="pp", name="wps")
            for _ in range(9):
                nc.tensor.matmul(wps[:], wmm[:, :128], wmm[:, :VC],
                                 start=True, stop=True)

            def load_hw(vc, half):
                hw = wpool.tile([128, 4, VC], F16, tag="hw",
                                name=f"hw{vc}{'ab'[half]}")
                nc.scalar.dma_start(
                    hw[:], hw_d.ap()[:, 4 * half:4 * half + 4, vc, :])
                hws[(vc, half)] = hw

            # interleave: xf0, hw0a, xf1..3, hw0b, xf4..7, hw1; then stream
            nc.sync.dma_start(xft[0][:], xf_d.ap()[:, 0, :])
            load_hw(0, 0)
            for i in range(1, 4):
                nc.sync.dma_start(xft[i][:], xf_d.ap()[:, i, :])
            load_hw(0, 1)
            for i in range(4, DT):
                nc.sync.dma_start(xft[i][:], xf_d.ap()[:, i, :])
            load_hw(1, 0)
            load_hw(1, 1)

            def drain(vc, tt, pp):
                ot = opool.tile([128, VC], F16, tag="ot")
                if tt % 2 == 0:
                    nc.scalar.copy(ot[:], pp[:])
                else:
                    nc.vector.tensor_copy(ot[:], pp[:])
                deng = (nc.sync, nc.scalar)[tt % 2]
                deng.dma_start(
                    lo_d.ap()[tt * 128:(tt + 1) * 128,
                              vc * VC:(vc + 1) * VC], ot[:])

            # vc 0: dt-major so compute starts as xf/hw tiles stream in
            pps = []
            for tt in range(TT):
                pp = ps.tile([128, VC], F32, tag="pp", name=f"pp0_{tt}")
                pps.append(pp)
            for dt in range(DT):
                hw = hws[(0, dt // 4)]
                for tt in range(TT):
                    nc.tensor.matmul(pps[tt][:],
                                     xft[dt][:, tt * 128:(tt + 1) * 128],
                                     hw[:, dt % 4, :],
                                     start=(dt == 0), stop=(dt == DT - 1))
                    if dt == DT - 1:
                        drain(0, tt, pps[tt])

            load_hw(2, 0)
            load_hw(2, 1)

            # vc 1+: tt-major — each PSUM bank is held only ~1.7us, so bank
            # recycling never lands on the PE critical path and stores spread
            for vc in range(1, NVC):
                if vc + 2 < NVC:
                    load_hw(vc + 2, 0)
                    load_hw(vc + 2, 1)
                for tt in range(TT):
                    pp = ps.tile([128, VC], F32, tag="pp", name=f"pp{vc}_{tt}")
                    for dt in range(DT):
                        nc.tensor.matmul(pp[:],
                                         xft[dt][:, tt * 128:(tt + 1) * 128],
                                         hws[(vc, dt // 4)][:, dt % 4, :],
                                         start=(dt == 0), stop=(dt == DT - 1))
                    drain(vc, tt, pp)
    nc.compile()
    return nc


_PROGS = {}


def _prog(name):
    if name not in _PROGS:
        _PROGS[name] = {"A": _build_A,
                        "C1": lambda: _build_C(False),
                        "C2": lambda: _build_C(True),
                        "D": _build_D}[name]()
    return _PROGS[name]


# ---------------------------------------------------------------- host-side math


def _ln(x, w, b):
    m = x.mean(-1, keepdims=True, dtype=np.float32)
    v = ((x - m) ** 2).mean(-1, keepdims=True, dtype=np.float32)
    return ((x - m) / np.sqrt(v + np.float32(1e-5)) * w + b).astype(np.float32)


def _softmax(x, axis=-1):
    m = x.max(axis=axis, keepdims=True)
    e = np.exp(x - m)
    return e / e.sum(axis=axis, keepdims=True)


def _nw(xn, A, Bm, Wimp, Wr):
    """SSM scan + routing -> neuron weights [B, NC] (host, fp32)."""
    u = xn @ Bm                       # [B,S,SD]
    h = np.zeros((xn.shape[0], A.shape[0]), np.float32)
    for t in range(xn.shape[1]):
        h = h @ A + u[:, t]
    h_proj = h @ Wimp.T               # [B, D]
    imp = _softmax(np.einsum('bsd,bd->bs', xn, h_proj), axis=-1)
    pref = _softmax(xn @ Wr.T, axis=-1)
    nw = np.einsum('bs,bsn->bn', imp, pref)
    return (nw / (nw.sum(-1, keepdims=True) + np.float32(1e-8))).astype(np.float32)


def _tile_dmajor(a, dtype):
    """[rows(D-like), cols] -> [128, rows//128, cols] partition-major."""
    rows, cols = a.shape
    return np.ascontiguousarray(
        a.reshape(rows // 128, 128, cols).transpose(1, 0, 2), dtype=dtype)


_run_ncores = list(range(N_CORES))


def _run(name, in_maps):
    res = run_bass_kernel_spmd(_prog(name), in_maps, core_ids=_run_ncores)
    return res.results


def kernel(**inputs) -> np.ndarray:
    inp = {k: np.asarray(v) for k, v in inputs.items()}
    ids = inp['input_ids'].astype(np.int64)
    comp_f = inp['compress_neurons'].reshape(NC, -1).astype(np.float32)
    tri16 = np.triu(np.ones((128, 128), np.float16))
    kkT = np.ascontiguousarray(inp['knowledge_K'].T, np.float32)  # [R=128, NK]
    kv32 = _tile_dmajor(inp['knowledge_V'].astype(np.float32), np.float32)
    kv16 = kv32.astype(np.float16)

    x = (inp['tok_emb'][ids] + inp['pos_emb'][None, :ids.shape[1]]).astype(np.float32)

    for l in range(L):
        # ---- circuit (program A, batch x 4-head sharded) ----
        xn = _ln(x, inp['ln1_w'][l], inp['ln1_b'][l])
        nw = _nw(xn, inp['a_A'][l], inp['a_B'][l], inp['a_imp'][l], inp['a_router'][l])
        sc = (nw @ comp_f).reshape(B, D, R)
        eq = (nw @ inp['eQ'][l].reshape(NC, -1).astype(np.float32)).reshape(B, R, D)
        ek = (nw @ inp['eK'][l].reshape(NC, -1).astype(np.float32)).reshape(B, R, D)
        ev = (nw @ inp['eV'][l].reshape(NC, -1).astype(np.float32)).reshape(B, R, D)
        h = np.matmul(xn, sc)                       # [B, S, R] low-rank tokens
        Q = np.matmul(h, eq)                        # [B, S, D]
        K = np.matmul(h, ek)
        Vv = np.matmul(h, ev)
        woT = np.ascontiguousarray(inp['o_w'][l].T, dtype=np.float32)
        in_maps = []
        for c in range(N_CORES):
            b, hg = c // 4, c % 4
            hs = slice(256 * hg, 256 * hg + 256)
            qt = Q[b].T[hs].reshape(2, 128, S).transpose(1, 0, 2)
            kt = K[b].T[hs].reshape(2, 128, S).transpose(1, 0, 2)
            vt = np.ones((128, 4, 4, 65), np.float16)
            vt[:, :, :, :64] = (Vv[b][:, hs].reshape(4, 128, 4, 64)
                                .transpose(1, 0, 2, 3).astype(np.float16))
            in_maps.append({
                "qt": np.ascontiguousarray(qt, np.float16),
                "kt": np.ascontiguousarray(kt, np.float16),
                "vt": vt,
                "wo": np.ascontiguousarray(
                    woT[hs, :].reshape(2, 128, D).transpose(1, 0, 2), np.float32),
                "tri": tri16,
            })
        res = _run("A", in_maps)
        for b in range(B):
            acc = res[4 * b]["part"].astype(np.float32)
            for c in range(4 * b + 1, 4 * b + 4):
                acc = acc + res[c]["part"].astype(np.float32)
            x[b] += acc.reshape(D, S).T

        # ---- memory (program C1/C2, token-sharded) ----
        xn = _ln(x, inp['ln2_w'][l], inp['ln2_b'][l])
        nw = _nw(xn, inp['m_A'][l], inp['m_B'][l], inp['m_imp'][l], inp['m_router'][l])
        sc = (nw @ comp_f).reshape(B, D, R)
        # Q on host in fp64 (more accurate than any fp32 summation order)
        Qm = np.matmul(xn.astype(np.float64), sc.astype(np.float64))
        Qm = Qm.astype(np.float32)                  # [B, S, R]
        kv = kv32 if l == 0 else kv16
        in_maps = []
        for c in range(N_CORES):
            bc, s0 = c // 4, 128 * (c % 4)
            in_maps.append({
                "q": np.ascontiguousarray(Qm[bc, s0:s0 + 128].T),
                "kk": kkT,
                "kv": kv,
            })
        res = _run("C1" if l == 0 else "C2", in_maps)
        for c in range(N_CORES):
            bc, s0 = c // 4, 128 * (c % 4)
            x[bc, s0:s0 + 128] += res[c]["mo"].astype(np.float32)

    # ---- lm_head (program D, vocab-sharded) ----
    xf = _ln(x, inp['lnf_w'], inp['lnf_b'])
    xfT = np.concatenate([xf[b].T for b in range(B)], axis=1)  # [D, T]
    xfT16 = _tile_dmajor(xfT, np.float16)                      # [128, DT, T]
    hwT = np.ascontiguousarray(inp['head_w'].astype(np.float32).T)  # [D, V]
    in_maps = []
    for c in range(N_CORES):
        sl = hwT[:, VSL * c:VSL * (c + 1)]                     # [D, 4000]
        tiled = sl.reshape(DT, 128, NVC, VC).transpose(1, 0, 2, 3)
        in_maps.append({"xf": xfT16,
                        "hw": np.ascontiguousarray(tiled, np.float16)})
    res = _run("D", in_maps)
    logits = np.concatenate([res[c]["lo"].astype(np.float32)
                             for c in range(N_CORES)], axis=1)
    return logits.reshape(B, S, V)
